# revision 27
# baseline (speedup 1.0000x reference)
"""Trainium2 Bass kernel for fused GQA attention block (B=2, L=2048, D=2048,
H=16 q-heads, KV=4 kv-heads, HD=64, causal, QK-RMSNorm + RoPE).

Sharding (8 cores): core c -> batch b = c // 4, head-group g = c % 4
(query heads 4g..4g+3, kv head g). Each core computes its 4 heads'
attention and a partial output projection (256 of 1024 e-channels);
host sums the 4 partials per batch.

v2: bf16 matmul path, XBAR DMA transposes, causal q-slicing in attention,
PE mask-strip accumulate, rstd via ln/exp (single activation table),
fp16 output writeback.
"""

import os

import numpy as np
from ml_dtypes import bfloat16 as np_bf16

import bass_rust as _bass_rust
import concourse.bass as bass
import concourse.mybir as mybir
import concourse.tile as tile
from concourse import bacc
from concourse import bass_utils
from concourse.hw_specs import get_activation_tables
from concourse.masks import make_identity


class _PinnedBacc(bacc.Bacc):
    """Bacc that pins all activations to one act-func table.

    Square/Ln/Exp/Copy all live in natural_log_exp_and_others; the default
    greedy table choice alternates tables (Ln -> natural_log, Exp ->
    exp_and_others), inserting a 1.3us table load per switch. Emptying the
    other tables makes the selection pass pick the shared table with its
    original act_info index, so exactly one load is emitted."""

    _PIN_TABLE = "natural_log_exp_and_others"

    def insert_act_table_loads(self):
        has_activation = any(
            isinstance(i, mybir.InstActivation)
            for b in self.main_func.blocks
            for i in b.instructions
        )
        if not has_activation:
            return
        tables = [(k, v if k == self._PIN_TABLE else set())
                  for k, v in get_activation_tables(self.m.arch).items()]
        _bass_rust.insert_act_table_loads(self, tables)

F32 = mybir.dt.float32
F16 = mybir.dt.float16
BF16 = mybir.dt.bfloat16
AF = mybir.ActivationFunctionType
ALU = mybir.AluOpType

B, L, D = 2, 2048, 2048
H, KV, HD = 16, 4, 64
EPS = 1e-6
ROPE_BASE = 10000.0
N_CORES = 8
GQ = H // KV          # 4 query heads per core
LT = L // 128         # 16 l-tiles
DT = D // 128         # 16 d-tiles (contraction tiles for qkv proj)
TQ = 512              # q-chunk width for attention
NQC = L // TQ         # 4 q-chunks
NKB = L // 128        # 16 k-blocks
EW = (GQ + 2) * HD    # 384 qkv channels per core
EO = GQ * HD          # 256 output channels per core
G5 = GQ + 1           # norm groups (4 q heads + 1 k head)

KOPT_SCHED = os.environ.get("KOPT_SCHED", "ilv")


def _classify_mask(mask):
    """Per (kb, qt) block: 'skip' | 'full' | pattern index into mixed list.

    Patterns are transposed slices maskT[k0:k0+128, q0:q0+TQ]."""
    kinds = {}
    patterns = []
    pat_ids = {}
    deltas = {}
    for qt in range(NQC):
        for kb in range(NKB):
            sub = mask[qt * TQ:(qt + 1) * TQ, kb * 128:(kb + 1) * 128]
            if np.all(sub <= -1e8):
                kinds[(kb, qt)] = "skip"
            elif np.all(sub == 0.0):
                kinds[(kb, qt)] = "full"
            else:
                pt = np.ascontiguousarray(sub.T.astype(np.float32))
                key = pt.tobytes()
                if key not in pat_ids:
                    pat_ids[key] = len(patterns)
                    patterns.append(pt)
                kinds[(kb, qt)] = pat_ids[key]
                # affine (causal-boundary) pattern? keep iff q >= k
                delta = kb * 128 - qt * TQ
                kk = np.arange(128)[:, None]
                qq = np.arange(TQ)[None, :]
                causal = np.where(qq >= kk + delta, 0.0, -1e9).astype(np.float32)
                deltas[(kb, qt)] = delta if np.array_equal(pt, causal) else None
    return kinds, patterns, deltas


def _build_program(kinds, n_mixed, repeat=1, deltas=None, W_FOLDED=False):
    nc = _PinnedBacc("TRN2", target_bir_lowering=False, debug=False,
                     enable_asserts=False, num_devices=N_CORES)
    deltas = deltas or {}
    # causal fast path: every mixed pattern has an affine delta
    causal = all(deltas.get(k) is not None
                 for k, v in kinds.items() if not isinstance(v, str))

    # DRAM I/O (per core). Host pre-tiles everything into DMA-friendly layouts.
    xT = nc.dram_tensor("xT", [LT, D, 128], BF16, kind="ExternalInput").ap()
    wqkT = nc.dram_tensor("wqkT", [D, EW], BF16, kind="ExternalInput").ap()
    woT = nc.dram_tensor("woT", [EO, D], BF16, kind="ExternalInput").ap()
    cos2 = nc.dram_tensor("cos2", [128, LT * 32], BF16, kind="ExternalInput").ap()
    sin2 = nc.dram_tensor("sin2", [128, LT * 32], BF16, kind="ExternalInput").ap()
    if not W_FOLDED:
        qw = nc.dram_tensor("qw", [128, G5 * HD], BF16, kind="ExternalInput").ap()
    if n_mixed:
        if causal:
            mblk = nc.dram_tensor("mblk", [128, n_mixed * 128], BF16,
                                  kind="ExternalInput").ap()
        else:
            mblk = nc.dram_tensor("mblk", [128, n_mixed * TQ], BF16,
                                  kind="ExternalInput").ap()
    y = nc.dram_tensor("y", [L, D], F16, kind="ExternalOutput").ap()

    with tile.TileContext(nc) as tc:
        with (
            tc.tile_pool(name="consts", bufs=1) as consts,
            tc.tile_pool(name="wpool", bufs=1) as wpool,
            tc.tile_pool(name="xcolp", bufs=4) as xcolp,
            tc.tile_pool(name="work", bufs=4) as work,
            tc.tile_pool(name="nrm", bufs=4) as nrm,
            tc.tile_pool(name="persist", bufs=1) as persist,
            tc.tile_pool(name="pp", bufs=8) as pp,
            tc.tile_pool(name="zp", bufs=3) as zp,
            tc.tile_pool(name="ps_a", bufs=2, space="PSUM") as ps_a,
            tc.tile_pool(name="ps_b", bufs=2, space="PSUM") as ps_b,
            tc.tile_pool(name="ps_sc", bufs=2, space="PSUM") as ps_sc,
        ):
            # ---- constants ----
            identf = consts.tile([128, 128], F32, tag="identf")
            make_identity(nc, identf[:])
            ident = consts.tile([128, 128], BF16, tag="ident")
            nc.vector.tensor_copy(ident[:], identf[:])
            cos_sb = consts.tile([128, LT * 32], BF16, tag="cos")
            sin_sb = consts.tile([128, LT * 32], BF16, tag="sin")
            nc.sync.dma_start(cos_sb[:], cos2[:])
            nc.sync.dma_start(sin_sb[:], sin2[:])
            # per-group ln() bias tiles for rstd = exp(-.5*ln(ss*s + b))
            # q groups: rstd = (w0q/sqrt(HD)) / sqrt(ss/HD + eps)
            #         = exp(-.5 * ln(ss * sq + bq))
            bq_sb = consts.tile([128, 1], F32, tag="bq")
            bk_sb = consts.tile([128, 1], F32, tag="bk")
            # constants depend on fold coefficients; host passes them via
            # module-level attributes set in _host_prep (W_FOLDED only).
            cq, ck = _FOLD_COEFS
            sq_scale = 1.0 / (HD * cq * cq)
            bq_val = EPS / (cq * cq)
            sk_scale = 1.0 / (HD * ck * ck)
            bk_val = EPS / (ck * ck)
            nc.vector.memset(bq_sb[:], bq_val)
            nc.vector.memset(bk_sb[:], bk_val)
            if not W_FOLDED:
                w5_sb = consts.tile([128, G5 * HD], BF16, tag="w5")
                nc.sync.dma_start(w5_sb[:], qw[:])
            if n_mixed:
                mwid = 128 if causal else TQ
                mb_sb = consts.tile([128, n_mixed * mwid], BF16, tag="mb")
                nc.sync.dma_start(mb_sb[:], mblk[:])

            # ---- weights (bf16) ----
            wqk_sb = []
            for dt_i in range(DT):
                w = wpool.tile([128, EW], BF16, tag=f"wqk{dt_i}")
                nc.sync.dma_start(w[:], wqkT[dt_i * 128:(dt_i + 1) * 128, :])
                wqk_sb.append(w)
            wo_sb = []
            for et in range(2):
                w = wpool.tile([128, D], BF16, tag=f"wo{et}")
                nc.sync.dma_start(w[:], woT[et * 128:(et + 1) * 128, :])
                wo_sb.append(w)

            # ---- persistent attention operands ----
            # Q^T head pairs stacked on partitions: qpair rows 0-63 = head 2i,
            # rows 64-127 = head 2i+1 (direct XBAR transpose layout).
            # K^T duplicated on both halves. V-hat [tok part, 64 v | 64 ones].
            qpair2 = persist.tile([128, 2 * L], BF16, tag="qpair2")
            kt_sb = persist.tile([128, L], BF16, tag="kt")
            vt_sb = persist.tile([128, LT * 128], BF16, tag="vt")
            ones_sb = consts.tile([128, HD], BF16, tag="ones")
            nc.vector.memset(ones_sb[:], 1.0)
            for i in range(LT):
                nc.vector.tensor_copy(
                    vt_sb[:, i * 128 + HD:(i + 1) * 128], ones_sb[:])
            aot_sb = [persist.tile([128, L], BF16, tag=f"aot{et}",
                                   name=f"aot{et}")
                      for et in range(2)]

            # ================= Phase 1: QKV + RMSNorm + RoPE =================
            xcols = {}

            def prefetch_x(lt):
                if lt >= LT or lt in xcols:
                    return
                xcol = xcolp.tile([128, D], BF16, tag="xcol")
                # SWDGE queue (gpsimd): keeps the SP queue free for the
                # latency-critical XBAR transposes.
                nc.gpsimd.dma_start(
                    xcol[:].rearrange("p (t j) -> p t j", j=128),
                    xT[lt, :, :].rearrange("(t p) j -> p t j", p=128))
                xcols[lt] = xcol

            def emit_p1(lt):
                prefetch_x(lt)
                xcol = xcols.pop(lt)
                prefetch_x(lt + 1)
                prefetch_x(lt + 2)
                qkv_ps = ps_a.tile([128, 512], F32, tag="mm_a")
                qk = qkv_ps[:, 0:EW]
                for dt_i in range(DT):
                    nc.tensor.matmul(
                        qk, xcol[:, dt_i * 128:(dt_i + 1) * 128],
                        wqk_sb[dt_i][:],
                        start=(dt_i == 0), stop=(dt_i == DT - 1))
                # V copy right away so the qkv PSUM slot frees early
                nc.vector.tensor_copy(
                    vt_sb[:, lt * 128:lt * 128 + HD],
                    qk[:, G5 * HD:(G5 + 1) * HD])
                # RMS stats for 5 norm groups (4 q heads + 1 k head):
                # one batched Square then a per-group DVE reduction
                sq5 = work.tile([128, G5 * HD], BF16, tag="sq5")
                nc.scalar.activation(sq5[:], qk[:, 0:G5 * HD], AF.Square)
                ss = nrm.tile([128, 16], F32, tag="ss")
                nc.vector.tensor_reduce(
                    ss[:, 0:G5],
                    sq5[:].rearrange("p (h e) -> p h e", e=HD),
                    axis=mybir.AxisListType.X, op=ALU.add)
                # rstd = exp(-.5*ln(ss*s + b)); same act table as Exp/Square
                nc.scalar.activation(ss[:, 8:8 + GQ], ss[:, 0:GQ],
                                     AF.Ln, bias=bq_sb[:], scale=sq_scale)
                nc.scalar.activation(ss[:, 8 + GQ:8 + G5], ss[:, GQ:G5],
                                     AF.Ln, bias=bk_sb[:], scale=sk_scale)
                rstd = nrm.tile([128, 8], F32, tag="rstd")
                nc.scalar.activation(rstd[:, 0:G5], ss[:, 8:8 + G5],
                                     AF.Exp, scale=-0.5)

                # normalize: qn = qkv * rstd (broadcast over head dim)
                qn = work.tile([128, G5 * HD], BF16, tag="qn")
                nc.vector.tensor_tensor(
                    qn[:].rearrange("p (h e) -> p h e", e=HD),
                    qk[:, 0:G5 * HD].rearrange("p (h e) -> p h e", e=HD),
                    rstd[:, 0:G5, None].broadcast_to([128, G5, HD]),
                    op=ALU.mult)
                if not W_FOLDED:
                    nc.vector.tensor_tensor(qn[:], qn[:], w5_sb[:],
                                            op=ALU.mult)

                # RoPE on all 5 groups at once (bf16, DVE 2x mode)
                cs = cos_sb[:, lt * 32:(lt + 1) * 32]
                sn = sin_sb[:, lt * 32:(lt + 1) * 32]
                csq = cs[:, None, :].broadcast_to([128, G5, 32])
                snq = sn[:, None, :].broadcast_to([128, G5, 32])
                # rq layout: q0 q1 q2 q3 k k2 (k duplicated for kt transpose)
                rq = work.tile([128, (G5 + 1) * HD], BF16, tag="rq")
                rqv = rq[:, 0:G5 * HD].rearrange("p (h e) -> p h e", e=HD)
                qnv = qn[:].rearrange("p (h e) -> p h e", e=HD)
                t1 = work.tile([128, G5 * 32], BF16, tag="t1")
                t1v = t1[:].rearrange("p (h e) -> p h e", e=32)
                # low half: x1*cos - x2*sin
                nc.vector.tensor_tensor(t1v, qnv[:, :, 0:32], csq, op=ALU.mult)
                nc.vector.tensor_tensor(rqv[:, :, 0:32], qnv[:, :, 32:64], snq,
                                        op=ALU.mult)
                nc.vector.tensor_tensor(rqv[:, :, 0:32], t1v,
                                        rqv[:, :, 0:32], op=ALU.subtract)
                # high half: x1*sin + x2*cos
                nc.vector.tensor_tensor(t1v, qnv[:, :, 0:32], snq, op=ALU.mult)
                nc.vector.tensor_tensor(rqv[:, :, 32:64], qnv[:, :, 32:64], csq,
                                        op=ALU.mult)
                nc.vector.tensor_tensor(rqv[:, :, 32:64], t1v,
                                        rqv[:, :, 32:64], op=ALU.add)
                # duplicate k so one XBAR op yields kt on both halves
                nc.vector.tensor_copy(rq[:, G5 * HD:(G5 + 1) * HD],
                                      rq[:, GQ * HD:G5 * HD])

                # transposes via XBAR DMA (no PE, no DVE)
                for pr in range(2):
                    nc.sync.dma_start_transpose(
                        qpair2[:, pr * L + lt * 128:pr * L + (lt + 1) * 128],
                        rq[:, pr * 128:(pr + 1) * 128])
                nc.sync.dma_start_transpose(
                    kt_sb[:, lt * 128:(lt + 1) * 128],
                    rq[:, GQ * HD:(GQ + 2) * HD])

            # ================= Phase 2: attention =================
            # Per (pair, qc): both sub-heads' scores/exp/AV pipeline over kb
            # with per-sub 1-bank PSUM tiles; AV of step kb-1 is emitted
            # after the scores of kb so PE always has ready work while the
            # exp for kb runs.
            def p2_steps(qc):
                """Yield emission closures for one q-chunk, software-pipelined."""
                klist = [kb for kb in range(NKB) if kinds[(kb, qc)] != "skip"]
                if not klist:
                    return

                for pr in range(2):
                    qsl = qpair2[:, pr * L + qc * TQ:pr * L + (qc + 1) * TQ]
                    avs = []    # allocated lazily at first step execution
                    pend = []   # (kb, d, p0, p1) awaiting AV

                    def alloc_avs(avs=avs, qc=qc, pr=pr):
                        if not avs:
                            for s in range(2):
                                avs.append(ps_b.tile(
                                    [128, TQ], F32, tag="av",
                                    name=f"av{qc}_{pr}_{s}"))

                    def flush_av(pend=pend, avs=avs, last=False):
                        while pend and (last or len(pend) > 1):
                            kb0, d0, p_sb = pend.pop(0)
                            fin = last and not pend
                            for sub in range(2):
                                nc.tensor.matmul(
                                    avs[sub][:, d0:TQ],
                                    vt_sb[:, kb0 * 128:(kb0 + 1) * 128],
                                    p_sb[:, sub * TQ + d0:(sub + 1) * TQ],
                                    start=kb0 == klist[0], stop=fin,
                                    skip_group_check=True)

                    def step(kb, pr=pr, qsl=qsl, avs=avs, pend=pend,
                             alloc_avs=alloc_avs, flush_av=flush_av):
                        alloc_avs()
                        kind = kinds[(kb, qc)]
                        delta = deltas.get((kb, qc))
                        if kind == "full" or not causal:
                            d = 0
                        else:
                            d = max(delta, 0)
                        sc_ps = ps_sc.tile([128, 2 * TQ], F32, tag="sc")
                        for sub in range(2):
                            nc.tensor.matmul(
                                sc_ps[:, sub * TQ + d:(sub + 1) * TQ],
                                kt_sb[sub * 64:(sub + 1) * 64,
                                      kb * 128:(kb + 1) * 128],
                                qsl[sub * 64:(sub + 1) * 64, d:TQ],
                                start=True, stop=(kind == "full"),
                                skip_group_check=True)
                        if kind != "full":
                            if causal:
                                m_mv = mb_sb[:, kind * 128:(kind + 1) * 128]
                                moff, mw = d, 128
                            else:
                                m_mv = mb_sb[:, kind * TQ:(kind + 1) * TQ]
                                moff, mw = 0, TQ
                            for sub in range(2):
                                nc.tensor.matmul(
                                    sc_ps[:, sub * TQ + moff:
                                          sub * TQ + moff + mw],
                                    ident[:], m_mv,
                                    start=False, stop=True,
                                    skip_group_check=True)
                        flush_av()
                        p_sb = pp.tile([128, 2 * TQ], BF16, tag="p")
                        sc_view = sc_ps[:].rearrange(
                            "p (s q) -> p s q", q=TQ)[:, :, d:TQ]
                        p_view = p_sb[:].rearrange(
                            "p (s q) -> p s q", q=TQ)[:, :, d:TQ]
                        nc.scalar.activation(p_view, sc_view, AF.Exp)
                        pend.append((kb, d, p_sb))

                    for kb in klist:
                        yield lambda kb=kb, step=step: step(kb)

                    def finish(pr=pr, avs=avs, pend=pend, flush_av=flush_av):
                        flush_av(last=True)
                        for sub in range(2):
                            rec = work.tile([64, TQ], F32, tag="rec")
                            nc.vector.reciprocal(rec[:], avs[sub][64:128, :])
                            nc.vector.tensor_tensor(
                                aot_sb[pr][sub * 64:(sub + 1) * 64,
                                           qc * TQ:(qc + 1) * TQ],
                                avs[sub][0:64, :], rec[:], op=ALU.mult)

                    yield finish

            # ================= Phase 3: output projection =================
            def emit_p3(lt):
                zo = zp.tile([128, D], F16, tag="zo")
                for dc in range(4):
                    z_ps = ps_a.tile([128, 512], F32, tag="mm_a")
                    for et in range(2):
                        nc.tensor.matmul(
                            z_ps[:], aot_sb[et][:, lt * 128:(lt + 1) * 128],
                            wo_sb[et][:, dc * 512:(dc + 1) * 512],
                            start=(et == 0), stop=(et == 1))
                    zslice = zo[:, dc * 512:(dc + 1) * 512]
                    if dc % 2 == 0:
                        nc.vector.tensor_copy(zslice, z_ps[:])
                    else:
                        nc.scalar.copy(zslice, z_ps[:])
                nc.gpsimd.dma_start(y[lt * 128:(lt + 1) * 128, :], zo[:])

            def emit_body():
                if KOPT_SCHED == "seq":
                    for lt in range(LT):
                        emit_p1(lt)
                    for qc in range(NQC):
                        for s in p2_steps(qc):
                            s()
                    for lt in range(LT):
                        emit_p3(lt)
                else:
                    # interleaved: P1 tiles run ahead of P2 q-chunks; P3
                    # trails one q-chunk behind P2.
                    lt_per_qc = TQ // 128
                    for lt in range(lt_per_qc):
                        emit_p1(lt)
                    next_p1 = lt_per_qc
                    next_p3 = 0
                    for qc in range(NQC):
                        steps = list(p2_steps(qc))
                        p1f = []
                        p1_hi = min(LT, lt_per_qc * (qc + 3))
                        while next_p1 < p1_hi:
                            p1f.append(next_p1)
                            next_p1 += 1
                        p3f = []
                        p3_hi = lt_per_qc * qc
                        while next_p3 < p3_hi:
                            p3f.append(next_p3)
                            next_p3 += 1
                        fillers = ([("p1", i) for i in p1f]
                                   + [("p3", i) for i in p3f])
                        nf, ns = len(fillers), max(len(steps), 1)
                        fi = 0
                        for si, s in enumerate(steps):
                            s()
                            want = (si + 1) * nf // ns
                            while fi < want:
                                kind, idx = fillers[fi]
                                (emit_p1 if kind == "p1" else emit_p3)(idx)
                                fi += 1
                        while fi < nf:
                            kind, idx = fillers[fi]
                            (emit_p1 if kind == "p1" else emit_p3)(idx)
                            fi += 1
                    while next_p3 < LT:
                        emit_p3(next_p3)
                        next_p3 += 1

            if repeat > 1:
                with tc.For_i(0, repeat, 1):
                    emit_body()
            else:
                emit_body()

    nc.compile()
    return nc


_PROGRAM_CACHE = {}
_FOLD_COEFS = (HD ** -0.5, 1.0)


def _get_program(kinds, n_mixed, repeat=1, deltas=None, W_FOLDED=False):
    key = (tuple(sorted(kinds.items())), n_mixed, repeat, W_FOLDED,
           _FOLD_COEFS, KOPT_SCHED)
    if key not in _PROGRAM_CACHE:
        _PROGRAM_CACHE[key] = _build_program(kinds, n_mixed, repeat, deltas,
                                             W_FOLDED)
    return _PROGRAM_CACHE[key]


def _host_prep(x, W_qkv, W_out, q_norm_w, k_norm_w, mask):
    global _FOLD_COEFS
    kinds, patterns, deltas = _classify_mask(np.asarray(mask))
    n_mixed = len(patterns)
    assert n_mixed <= 12, f"too many unique mask patterns: {n_mixed}"
    causal = all(deltas.get(k) is not None
                 for k, v in kinds.items() if not isinstance(v, str))

    # RoPE tables, tiled [128, LT*32]: cos2[p, lt*32+j] = cos((lt*128+p)*freq_j)
    j = np.arange(0, HD, 2, dtype=np.float32)
    freqs = (ROPE_BASE ** (-j / HD)).astype(np.float32)
    pos = np.arange(L, dtype=np.float32)
    theta = pos[:, None] * freqs[None, :]
    cosf = np.cos(theta).astype(np.float32)     # [L, 32]
    sinf = np.sin(theta).astype(np.float32)
    cos2 = np.ascontiguousarray(
        cosf.reshape(LT, 128, 32).transpose(1, 0, 2).reshape(128, LT * 32)
    ).astype(np_bf16)
    sin2 = np.ascontiguousarray(
        sinf.reshape(LT, 128, 32).transpose(1, 0, 2).reshape(128, LT * 32)
    ).astype(np_bf16)

    scale = np.float32(HD ** -0.5)
    qwv = np.asarray(q_norm_w, np.float32)
    kwv = np.asarray(k_norm_w, np.float32)
    # uniform norm weights fold into the rstd ln/exp constants
    w_folded = bool(np.all(qwv == qwv[0]) and np.all(kwv == kwv[0]))
    if w_folded:
        _FOLD_COEFS = (float(qwv[0]) * float(scale), float(kwv[0]))
        qw_rep = None
    else:
        _FOLD_COEFS = (float(scale), 1.0)
        w5 = np.concatenate([np.tile(qwv, GQ), kwv]).astype(np.float32)
        qw_rep = np.tile(w5[None, :], (128, 1)).astype(np_bf16)

    if n_mixed:
        if causal:
            strips = []
            for pi, pt in enumerate(patterns):
                dlist = [d for k, d in deltas.items()
                         if kinds.get(k) == pi and d is not None]
                d = max(dlist[0], 0)
                strips.append(pt[:, d:d + 128])
            mb = np.concatenate(strips, axis=1).astype(np_bf16)
        else:
            mb = np.concatenate(patterns, axis=1).astype(np_bf16)
    else:
        mb = None

    in_maps = []
    for c in range(N_CORES):
        b, g = divmod(c, KV)
        xb = np.asarray(x[b], np.float32)
        xTt = np.ascontiguousarray(
            xb.reshape(LT, 128, D).transpose(0, 2, 1)).astype(np_bf16)
        rows = np.r_[g * GQ * HD:(g + 1) * GQ * HD,
                     (H + g) * HD:(H + g + 1) * HD,
                     (H + KV + g) * HD:(H + KV + g + 1) * HD]
        wqkT = np.ascontiguousarray(
            np.asarray(W_qkv, np.float32)[rows].T).astype(np_bf16)
        cols = np.arange(g * GQ * HD, (g + 1) * GQ * HD)
        woT = np.ascontiguousarray(
            np.asarray(W_out, np.float32)[:, cols].T).astype(np_bf16)
        m = {"xT": xTt, "wqkT": wqkT, "woT": woT,
             "cos2": cos2, "sin2": sin2}
        if qw_rep is not None:
            m["qw"] = qw_rep
        if mb is not None:
            m["mblk"] = mb
        in_maps.append(m)
    return kinds, n_mixed, in_maps, deltas, w_folded


def kernel(x, W_qkv, W_out, q_norm_w, k_norm_w, mask):
    kinds, n_mixed, in_maps, deltas, wf = _host_prep(x, W_qkv, W_out,
                                                     q_norm_w, k_norm_w, mask)
    nc = _get_program(kinds, n_mixed, deltas=deltas, W_FOLDED=wf)
    res = bass_utils.run_bass_kernel_spmd(nc, in_maps,
                                          core_ids=list(range(N_CORES)))
    out = np.zeros((B, L, D), dtype=np.float32)
    for c in range(N_CORES):
        b = c // KV
        out[b] += res.results[c]["y"].astype(np.float32)
    return out


# revision 31
# speedup vs baseline: 1.3080x; 1.3080x over previous
"""Trainium2 Bass kernel for fused GQA attention block (B=2, L=2048, D=2048,
H=16 q-heads, KV=4 kv-heads, HD=64, causal, QK-RMSNorm + RoPE).

Sharding (8 cores): core c -> batch b = c // 4, head-group g = c % 4
(query heads 4g..4g+3, kv head g). Each core computes its 4 heads'
attention and a partial output projection (256 of 1024 e-channels);
host sums the 4 partials per batch.

v2: bf16 matmul path, XBAR DMA transposes, causal q-slicing in attention,
PE mask-strip accumulate, rstd via ln/exp (single activation table),
fp16 output writeback.
"""

import os

import numpy as np
from ml_dtypes import bfloat16 as np_bf16

import bass_rust as _bass_rust
import concourse.bass as bass
import concourse.mybir as mybir
import concourse.tile as tile
from concourse import bacc
from concourse import bass_utils
from concourse.hw_specs import get_activation_tables
from concourse.masks import make_identity


class _PinnedBacc(bacc.Bacc):
    """Bacc that pins all activations to one act-func table.

    Square/Ln/Exp/Copy all live in natural_log_exp_and_others; the default
    greedy table choice alternates tables (Ln -> natural_log, Exp ->
    exp_and_others), inserting a 1.3us table load per switch. Emptying the
    other tables makes the selection pass pick the shared table with its
    original act_info index, so exactly one load is emitted."""

    _PIN_TABLE = "natural_log_exp_and_others"

    def insert_act_table_loads(self):
        has_activation = any(
            isinstance(i, mybir.InstActivation)
            for b in self.main_func.blocks
            for i in b.instructions
        )
        if not has_activation:
            return
        tables = [(k, v if k == self._PIN_TABLE else set())
                  for k, v in get_activation_tables(self.m.arch).items()]
        _bass_rust.insert_act_table_loads(self, tables)

F32 = mybir.dt.float32
F16 = mybir.dt.float16
BF16 = mybir.dt.bfloat16
AF = mybir.ActivationFunctionType
ALU = mybir.AluOpType

B, L, D = 2, 2048, 2048
H, KV, HD = 16, 4, 64
EPS = 1e-6
ROPE_BASE = 10000.0
N_CORES = 8
GQ = H // KV          # 4 query heads per core
LT = L // 128         # 16 l-tiles
DT = D // 128         # 16 d-tiles (contraction tiles for qkv proj)
TQ = 512              # q-chunk width for attention
NQC = L // TQ         # 4 q-chunks
NKB = L // 128        # 16 k-blocks
EW = (GQ + 2) * HD    # 384 qkv channels per core
EO = GQ * HD          # 256 output channels per core
G5 = GQ + 1           # norm groups (4 q heads + 1 k head)

KOPT_SCHED = os.environ.get("KOPT_SCHED", "ilv")


def _classify_mask(mask):
    """Per (kb, qt) block: 'skip' | 'full' | pattern index into mixed list.

    Patterns are transposed slices maskT[k0:k0+128, q0:q0+TQ]."""
    kinds = {}
    patterns = []
    pat_ids = {}
    deltas = {}
    for qt in range(NQC):
        for kb in range(NKB):
            sub = mask[qt * TQ:(qt + 1) * TQ, kb * 128:(kb + 1) * 128]
            if np.all(sub <= -1e8):
                kinds[(kb, qt)] = "skip"
            elif np.all(sub == 0.0):
                kinds[(kb, qt)] = "full"
            else:
                pt = np.ascontiguousarray(sub.T.astype(np.float32))
                key = pt.tobytes()
                if key not in pat_ids:
                    pat_ids[key] = len(patterns)
                    patterns.append(pt)
                kinds[(kb, qt)] = pat_ids[key]
                # affine (causal-boundary) pattern? keep iff q >= k
                delta = kb * 128 - qt * TQ
                kk = np.arange(128)[:, None]
                qq = np.arange(TQ)[None, :]
                causal = np.where(qq >= kk + delta, 0.0, -1e9).astype(np.float32)
                deltas[(kb, qt)] = delta if np.array_equal(pt, causal) else None
    return kinds, patterns, deltas


def _build_program(kinds, n_mixed, repeat=1, deltas=None, W_FOLDED=False):
    nc = _PinnedBacc("TRN2", target_bir_lowering=False, debug=False,
                     enable_asserts=False, num_devices=N_CORES)
    deltas = deltas or {}
    # causal fast path: every mixed pattern has an affine delta
    causal = all(deltas.get(k) is not None
                 for k, v in kinds.items() if not isinstance(v, str))

    # DRAM I/O (per core). Host pre-tiles everything into DMA-friendly layouts.
    # xT[lt] is the exact [128, D] SBUF image: xT[lt][p, t*128+j] =
    # x[b, lt*128+j, t*128+p], so the load is one contiguous-line DMA.
    xT = nc.dram_tensor("xT", [LT, 128, D], BF16, kind="ExternalInput").ap()
    wqkT = nc.dram_tensor("wqkT", [D, EW], BF16, kind="ExternalInput").ap()
    woT = nc.dram_tensor("woT", [EO, D], BF16, kind="ExternalInput").ap()
    cos2 = nc.dram_tensor("cos2", [128, LT * 32], BF16, kind="ExternalInput").ap()
    sin2 = nc.dram_tensor("sin2", [128, LT * 32], BF16, kind="ExternalInput").ap()
    if not W_FOLDED:
        qw = nc.dram_tensor("qw", [128, G5 * HD], BF16, kind="ExternalInput").ap()
    if n_mixed:
        if causal:
            mblk = nc.dram_tensor("mblk", [128, n_mixed * 128], BF16,
                                  kind="ExternalInput").ap()
        else:
            mblk = nc.dram_tensor("mblk", [128, n_mixed * TQ], BF16,
                                  kind="ExternalInput").ap()
    y = nc.dram_tensor("y", [L, D], F16, kind="ExternalOutput").ap()

    with tile.TileContext(nc) as tc:
        with (
            tc.tile_pool(name="consts", bufs=1) as consts,
            tc.tile_pool(name="wpool", bufs=1) as wpool,
            tc.tile_pool(name="xcolp", bufs=4) as xcolp,
            tc.tile_pool(name="work", bufs=4) as work,
            tc.tile_pool(name="nrm", bufs=4) as nrm,
            tc.tile_pool(name="persist", bufs=1) as persist,
            tc.tile_pool(name="pp", bufs=10) as pp,
            tc.tile_pool(name="zp", bufs=3) as zp,
            tc.tile_pool(name="ps_a", bufs=2, space="PSUM") as ps_a,
            tc.tile_pool(name="ps_b", bufs=2, space="PSUM") as ps_b,
            tc.tile_pool(name="ps_sc", bufs=2, space="PSUM") as ps_sc,
        ):
            # ---- constants ----
            identf = consts.tile([128, 128], F32, tag="identf")
            make_identity(nc, identf[:])
            ident = consts.tile([128, 128], BF16, tag="ident")
            nc.vector.tensor_copy(ident[:], identf[:])
            cos_sb = consts.tile([128, LT * 32], BF16, tag="cos")
            sin_sb = consts.tile([128, LT * 32], BF16, tag="sin")
            nc.sync.dma_start(cos_sb[:], cos2[:])
            nc.sync.dma_start(sin_sb[:], sin2[:])
            # per-group ln() bias tiles for rstd = exp(-.5*ln(ss*s + b))
            # q groups: rstd = (w0q/sqrt(HD)) / sqrt(ss/HD + eps)
            #         = exp(-.5 * ln(ss * sq + bq))
            bq_sb = consts.tile([128, 1], F32, tag="bq")
            bk_sb = consts.tile([128, 1], F32, tag="bk")
            # constants depend on fold coefficients; host passes them via
            # module-level attributes set in _host_prep (W_FOLDED only).
            cq, ck = _FOLD_COEFS
            sq_scale = 1.0 / (HD * cq * cq)
            bq_val = EPS / (cq * cq)
            sk_scale = 1.0 / (HD * ck * ck)
            bk_val = EPS / (ck * ck)
            nc.vector.memset(bq_sb[:], bq_val)
            nc.vector.memset(bk_sb[:], bk_val)
            if not W_FOLDED:
                w5_sb = consts.tile([128, G5 * HD], BF16, tag="w5")
                nc.sync.dma_start(w5_sb[:], qw[:])
            if n_mixed:
                mwid = 128 if causal else TQ
                mb_sb = consts.tile([128, n_mixed * mwid], BF16, tag="mb")
                nc.sync.dma_start(mb_sb[:], mblk[:])

            # ---- weights (bf16) ----
            wqk_sb = []
            for dt_i in range(DT):
                w = wpool.tile([128, EW], BF16, tag=f"wqk{dt_i}")
                nc.sync.dma_start(w[:], wqkT[dt_i * 128:(dt_i + 1) * 128, :])
                wqk_sb.append(w)
            wo_sb = []
            for et in range(2):
                w = wpool.tile([128, D], BF16, tag=f"wo{et}")
                nc.sync.dma_start(w[:], woT[et * 128:(et + 1) * 128, :])
                wo_sb.append(w)

            # ---- persistent attention operands ----
            # Q^T head pairs stacked on partitions: qpair rows 0-63 = head 2i,
            # rows 64-127 = head 2i+1 (direct XBAR transpose layout).
            # K^T duplicated on both halves. V-hat [tok part, 64 v | 64 ones].
            qpair2 = persist.tile([128, 2 * L], BF16, tag="qpair2")
            kt_sb = persist.tile([128, L], BF16, tag="kt")
            vt_sb = persist.tile([128, LT * 128], BF16, tag="vt")
            ones_sb = consts.tile([128, HD], BF16, tag="ones")
            nc.vector.memset(ones_sb[:], 1.0)
            for i in range(LT):
                nc.vector.tensor_copy(
                    vt_sb[:, i * 128 + HD:(i + 1) * 128], ones_sb[:])
            aot_sb = [persist.tile([128, L], BF16, tag=f"aot{et}",
                                   name=f"aot{et}")
                      for et in range(2)]

            # ================= Phase 1: QKV + RMSNorm + RoPE =================
            xcols = {}

            def prefetch_x(lt):
                if lt >= LT or lt in xcols:
                    return
                xcol = xcolp.tile([128, D], BF16, tag="xcol")
                # SWDGE queue (gpsimd): keeps the SP queue free for the
                # latency-critical XBAR transposes.
                nc.gpsimd.dma_start(xcol[:], xT[lt, :, :])
                xcols[lt] = xcol

            def emit_p1(lt):
                prefetch_x(lt)
                xcol = xcols.pop(lt)
                prefetch_x(lt + 1)
                prefetch_x(lt + 2)
                qkv_ps = ps_a.tile([128, 512], F32, tag="mm_a")
                qk = qkv_ps[:, 0:EW]
                for dt_i in range(DT):
                    nc.tensor.matmul(
                        qk, xcol[:, dt_i * 128:(dt_i + 1) * 128],
                        wqk_sb[dt_i][:],
                        start=(dt_i == 0), stop=(dt_i == DT - 1))
                # V copy right away so the qkv PSUM slot frees early
                nc.vector.tensor_copy(
                    vt_sb[:, lt * 128:lt * 128 + HD],
                    qk[:, G5 * HD:(G5 + 1) * HD])
                # RMS stats for 5 norm groups (4 q heads + 1 k head):
                # one batched Square then a per-group DVE reduction
                sq5 = work.tile([128, G5 * HD], BF16, tag="sq5")
                nc.scalar.activation(sq5[:], qk[:, 0:G5 * HD], AF.Square)
                ss = nrm.tile([128, 16], F32, tag="ss")
                nc.vector.tensor_reduce(
                    ss[:, 0:G5],
                    sq5[:].rearrange("p (h e) -> p h e", e=HD),
                    axis=mybir.AxisListType.X, op=ALU.add)
                # rstd = exp(-.5*ln(ss*s + b)); same act table as Exp/Square
                nc.scalar.activation(ss[:, 8:8 + GQ], ss[:, 0:GQ],
                                     AF.Ln, bias=bq_sb[:], scale=sq_scale)
                nc.scalar.activation(ss[:, 8 + GQ:8 + G5], ss[:, GQ:G5],
                                     AF.Ln, bias=bk_sb[:], scale=sk_scale)
                rstd = nrm.tile([128, 8], F32, tag="rstd")
                nc.scalar.activation(rstd[:, 0:G5], ss[:, 8:8 + G5],
                                     AF.Exp, scale=-0.5)

                # normalize: qn = qkv * rstd (broadcast over head dim)
                qn = work.tile([128, G5 * HD], BF16, tag="qn")
                nc.vector.tensor_tensor(
                    qn[:].rearrange("p (h e) -> p h e", e=HD),
                    qk[:, 0:G5 * HD].rearrange("p (h e) -> p h e", e=HD),
                    rstd[:, 0:G5, None].broadcast_to([128, G5, HD]),
                    op=ALU.mult)
                if not W_FOLDED:
                    nc.vector.tensor_tensor(qn[:], qn[:], w5_sb[:],
                                            op=ALU.mult)

                # RoPE on all 5 groups at once (bf16, DVE 2x mode)
                cs = cos_sb[:, lt * 32:(lt + 1) * 32]
                sn = sin_sb[:, lt * 32:(lt + 1) * 32]
                csq = cs[:, None, :].broadcast_to([128, G5, 32])
                snq = sn[:, None, :].broadcast_to([128, G5, 32])
                # rq layout: q0 q1 q2 q3 k k2 (k duplicated for kt transpose)
                rq = work.tile([128, (G5 + 1) * HD], BF16, tag="rq")
                rqv = rq[:, 0:G5 * HD].rearrange("p (h e) -> p h e", e=HD)
                qnv = qn[:].rearrange("p (h e) -> p h e", e=HD)
                t1 = work.tile([128, G5 * 32], BF16, tag="t1")
                t1v = t1[:].rearrange("p (h e) -> p h e", e=32)
                # low half: x1*cos - x2*sin
                nc.vector.tensor_tensor(t1v, qnv[:, :, 0:32], csq, op=ALU.mult)
                nc.vector.tensor_tensor(rqv[:, :, 0:32], qnv[:, :, 32:64], snq,
                                        op=ALU.mult)
                nc.vector.tensor_tensor(rqv[:, :, 0:32], t1v,
                                        rqv[:, :, 0:32], op=ALU.subtract)
                # high half: x1*sin + x2*cos
                nc.vector.tensor_tensor(t1v, qnv[:, :, 0:32], snq, op=ALU.mult)
                nc.vector.tensor_tensor(rqv[:, :, 32:64], qnv[:, :, 32:64], csq,
                                        op=ALU.mult)
                nc.vector.tensor_tensor(rqv[:, :, 32:64], t1v,
                                        rqv[:, :, 32:64], op=ALU.add)
                # duplicate k so one XBAR op yields kt on both halves
                nc.vector.tensor_copy(rq[:, G5 * HD:(G5 + 1) * HD],
                                      rq[:, GQ * HD:G5 * HD])

                # transposes via XBAR DMA (no PE, no DVE)
                for pr in range(2):
                    nc.sync.dma_start_transpose(
                        qpair2[:, pr * L + lt * 128:pr * L + (lt + 1) * 128],
                        rq[:, pr * 128:(pr + 1) * 128])
                nc.sync.dma_start_transpose(
                    kt_sb[:, lt * 128:(lt + 1) * 128],
                    rq[:, GQ * HD:(GQ + 2) * HD])

            # ================= Phase 2: attention =================
            # Per (pair, qc): both sub-heads' scores/exp/AV pipeline over kb
            # with per-sub 1-bank PSUM tiles; AV of step kb-1 is emitted
            # after the scores of kb so PE always has ready work while the
            # exp for kb runs.
            def p2_steps(qc):
                """Yield emission closures for one q-chunk, software-pipelined."""
                klist = [kb for kb in range(NKB) if kinds[(kb, qc)] != "skip"]
                if not klist:
                    return

                for pr in range(2):
                    qsl = qpair2[:, pr * L + qc * TQ:pr * L + (qc + 1) * TQ]
                    avs = []    # allocated lazily at first step execution
                    pend = []   # (kb, d, p0, p1) awaiting AV

                    def alloc_avs(avs=avs, qc=qc, pr=pr):
                        if not avs:
                            for s in range(2):
                                avs.append(ps_b.tile(
                                    [128, TQ], F32, tag="av",
                                    name=f"av{qc}_{pr}_{s}"))

                    def flush_av(pend=pend, avs=avs, last=False):
                        while pend and (last or len(pend) > 1):
                            kb0, d0, p_sb = pend.pop(0)
                            fin = last and not pend
                            for sub in range(2):
                                nc.tensor.matmul(
                                    avs[sub][:, d0:TQ],
                                    vt_sb[:, kb0 * 128:(kb0 + 1) * 128],
                                    p_sb[:, sub * TQ + d0:(sub + 1) * TQ],
                                    start=kb0 == klist[0], stop=fin,
                                    skip_group_check=True)

                    def step(kb, pr=pr, qsl=qsl, avs=avs, pend=pend,
                             alloc_avs=alloc_avs, flush_av=flush_av):
                        alloc_avs()
                        kind = kinds[(kb, qc)]
                        delta = deltas.get((kb, qc))
                        if kind == "full" or not causal:
                            d = 0
                        else:
                            d = max(delta, 0)
                        sc_ps = ps_sc.tile([128, 2 * TQ], F32, tag="sc")
                        for sub in range(2):
                            nc.tensor.matmul(
                                sc_ps[:, sub * TQ + d:(sub + 1) * TQ],
                                kt_sb[sub * 64:(sub + 1) * 64,
                                      kb * 128:(kb + 1) * 128],
                                qsl[sub * 64:(sub + 1) * 64, d:TQ],
                                start=True, stop=(kind == "full"),
                                skip_group_check=True)
                        if kind != "full":
                            if causal:
                                m_mv = mb_sb[:, kind * 128:(kind + 1) * 128]
                                moff, mw = d, 128
                            else:
                                m_mv = mb_sb[:, kind * TQ:(kind + 1) * TQ]
                                moff, mw = 0, TQ
                            for sub in range(2):
                                nc.tensor.matmul(
                                    sc_ps[:, sub * TQ + moff:
                                          sub * TQ + moff + mw],
                                    ident[:], m_mv,
                                    start=False, stop=True,
                                    skip_group_check=True)
                        flush_av()
                        p_sb = pp.tile([128, 2 * TQ], BF16, tag="p")
                        sc_view = sc_ps[:].rearrange(
                            "p (s q) -> p s q", q=TQ)[:, :, d:TQ]
                        p_view = p_sb[:].rearrange(
                            "p (s q) -> p s q", q=TQ)[:, :, d:TQ]
                        nc.scalar.activation(p_view, sc_view, AF.Exp)
                        pend.append((kb, d, p_sb))

                    for kb in klist:
                        yield lambda kb=kb, step=step: step(kb)

                    def finish(pr=pr, avs=avs, pend=pend, flush_av=flush_av):
                        flush_av(last=True)
                        for sub in range(2):
                            rec = work.tile([64, TQ], F32, tag="rec")
                            nc.vector.reciprocal(rec[:], avs[sub][64:128, :])
                            nc.vector.tensor_tensor(
                                aot_sb[pr][sub * 64:(sub + 1) * 64,
                                           qc * TQ:(qc + 1) * TQ],
                                avs[sub][0:64, :], rec[:], op=ALU.mult)

                    yield finish

            # ================= Phase 3: output projection =================
            def emit_p3(lt):
                zo = zp.tile([128, D], F16, tag="zo")
                for dc in range(4):
                    z_ps = ps_a.tile([128, 512], F32, tag="mm_a")
                    for et in range(2):
                        nc.tensor.matmul(
                            z_ps[:], aot_sb[et][:, lt * 128:(lt + 1) * 128],
                            wo_sb[et][:, dc * 512:(dc + 1) * 512],
                            start=(et == 0), stop=(et == 1))
                    zslice = zo[:, dc * 512:(dc + 1) * 512]
                    if dc % 2 == 0:
                        nc.vector.tensor_copy(zslice, z_ps[:])
                    else:
                        nc.scalar.copy(zslice, z_ps[:])
                nc.gpsimd.dma_start(y[lt * 128:(lt + 1) * 128, :], zo[:])

            def emit_body():
                if KOPT_SCHED == "seq":
                    for lt in range(LT):
                        emit_p1(lt)
                    for qc in range(NQC):
                        for s in p2_steps(qc):
                            s()
                    for lt in range(LT):
                        emit_p3(lt)
                else:
                    # interleaved: P1 tiles run ahead of P2 q-chunks; P3
                    # trails one q-chunk behind P2.
                    lt_per_qc = TQ // 128
                    for lt in range(lt_per_qc):
                        emit_p1(lt)
                    next_p1 = lt_per_qc
                    next_p3 = 0
                    for qc in range(NQC):
                        steps = list(p2_steps(qc))
                        p1f = []
                        p1_hi = min(LT, lt_per_qc * (qc + 3))
                        while next_p1 < p1_hi:
                            p1f.append(next_p1)
                            next_p1 += 1
                        p3f = []
                        p3_hi = lt_per_qc * qc
                        while next_p3 < p3_hi:
                            p3f.append(next_p3)
                            next_p3 += 1
                        fillers = ([("p1", i) for i in p1f]
                                   + [("p3", i) for i in p3f])
                        nf, ns = len(fillers), max(len(steps), 1)
                        fi = 0
                        for si, s in enumerate(steps):
                            s()
                            want = (si + 1) * nf // ns
                            while fi < want:
                                kind, idx = fillers[fi]
                                (emit_p1 if kind == "p1" else emit_p3)(idx)
                                fi += 1
                        while fi < nf:
                            kind, idx = fillers[fi]
                            (emit_p1 if kind == "p1" else emit_p3)(idx)
                            fi += 1
                    while next_p3 < LT:
                        emit_p3(next_p3)
                        next_p3 += 1

            if repeat > 1:
                with tc.For_i(0, repeat, 1):
                    emit_body()
            else:
                emit_body()

    nc.compile()
    return nc


_PROGRAM_CACHE = {}
_FOLD_COEFS = (HD ** -0.5, 1.0)


def _get_program(kinds, n_mixed, repeat=1, deltas=None, W_FOLDED=False):
    key = (tuple(sorted(kinds.items())), n_mixed, repeat, W_FOLDED,
           _FOLD_COEFS, KOPT_SCHED)
    if key not in _PROGRAM_CACHE:
        _PROGRAM_CACHE[key] = _build_program(kinds, n_mixed, repeat, deltas,
                                             W_FOLDED)
    return _PROGRAM_CACHE[key]


def _host_prep(x, W_qkv, W_out, q_norm_w, k_norm_w, mask):
    global _FOLD_COEFS
    kinds, patterns, deltas = _classify_mask(np.asarray(mask))
    n_mixed = len(patterns)
    assert n_mixed <= 12, f"too many unique mask patterns: {n_mixed}"
    causal = all(deltas.get(k) is not None
                 for k, v in kinds.items() if not isinstance(v, str))

    # RoPE tables, tiled [128, LT*32]: cos2[p, lt*32+j] = cos((lt*128+p)*freq_j)
    j = np.arange(0, HD, 2, dtype=np.float32)
    freqs = (ROPE_BASE ** (-j / HD)).astype(np.float32)
    pos = np.arange(L, dtype=np.float32)
    theta = pos[:, None] * freqs[None, :]
    cosf = np.cos(theta).astype(np.float32)     # [L, 32]
    sinf = np.sin(theta).astype(np.float32)
    cos2 = np.ascontiguousarray(
        cosf.reshape(LT, 128, 32).transpose(1, 0, 2).reshape(128, LT * 32)
    ).astype(np_bf16)
    sin2 = np.ascontiguousarray(
        sinf.reshape(LT, 128, 32).transpose(1, 0, 2).reshape(128, LT * 32)
    ).astype(np_bf16)

    scale = np.float32(HD ** -0.5)
    qwv = np.asarray(q_norm_w, np.float32)
    kwv = np.asarray(k_norm_w, np.float32)
    # uniform norm weights fold into the rstd ln/exp constants
    w_folded = bool(np.all(qwv == qwv[0]) and np.all(kwv == kwv[0]))
    if w_folded:
        _FOLD_COEFS = (float(qwv[0]) * float(scale), float(kwv[0]))
        qw_rep = None
    else:
        _FOLD_COEFS = (float(scale), 1.0)
        w5 = np.concatenate([np.tile(qwv, GQ), kwv]).astype(np.float32)
        qw_rep = np.tile(w5[None, :], (128, 1)).astype(np_bf16)

    if n_mixed:
        if causal:
            strips = []
            for pi, pt in enumerate(patterns):
                dlist = [d for k, d in deltas.items()
                         if kinds.get(k) == pi and d is not None]
                d = max(dlist[0], 0)
                strips.append(pt[:, d:d + 128])
            mb = np.concatenate(strips, axis=1).astype(np_bf16)
        else:
            mb = np.concatenate(patterns, axis=1).astype(np_bf16)
    else:
        mb = None

    in_maps = []
    for c in range(N_CORES):
        b, g = divmod(c, KV)
        xb = np.asarray(x[b], np.float32)
        # [LT, 128, D]: xTt[lt, p, t*128+j] = x[b, lt*128+j, t*128+p]
        xTt = np.ascontiguousarray(
            xb.reshape(LT, 128, DT, 128)        # [lt, j, t, p]
            .transpose(0, 3, 2, 1)              # [lt, p, t, j]
            .reshape(LT, 128, D)).astype(np_bf16)
        rows = np.r_[g * GQ * HD:(g + 1) * GQ * HD,
                     (H + g) * HD:(H + g + 1) * HD,
                     (H + KV + g) * HD:(H + KV + g + 1) * HD]
        wqkT = np.ascontiguousarray(
            np.asarray(W_qkv, np.float32)[rows].T).astype(np_bf16)
        cols = np.arange(g * GQ * HD, (g + 1) * GQ * HD)
        woT = np.ascontiguousarray(
            np.asarray(W_out, np.float32)[:, cols].T).astype(np_bf16)
        m = {"xT": xTt, "wqkT": wqkT, "woT": woT,
             "cos2": cos2, "sin2": sin2}
        if qw_rep is not None:
            m["qw"] = qw_rep
        if mb is not None:
            m["mblk"] = mb
        in_maps.append(m)
    return kinds, n_mixed, in_maps, deltas, w_folded


def kernel(x, W_qkv, W_out, q_norm_w, k_norm_w, mask):
    kinds, n_mixed, in_maps, deltas, wf = _host_prep(x, W_qkv, W_out,
                                                     q_norm_w, k_norm_w, mask)
    nc = _get_program(kinds, n_mixed, deltas=deltas, W_FOLDED=wf)
    res = bass_utils.run_bass_kernel_spmd(nc, in_maps,
                                          core_ids=list(range(N_CORES)))
    out = np.zeros((B, L, D), dtype=np.float32)
    for c in range(N_CORES):
        b = c // KV
        out[b] += res.results[c]["y"].astype(np.float32)
    return out


# revision 36
# speedup vs baseline: 1.3307x; 1.0173x over previous
"""Trainium2 Bass kernel for fused GQA attention block (B=2, L=2048, D=2048,
H=16 q-heads, KV=4 kv-heads, HD=64, causal, QK-RMSNorm + RoPE).

Sharding (8 cores): core c -> batch b = c // 4, head-group g = c % 4
(query heads 4g..4g+3, kv head g). Each core computes its 4 heads'
attention and a partial output projection (256 of 1024 e-channels);
host sums the 4 partials per batch.

v2: bf16 matmul path, XBAR DMA transposes, causal q-slicing in attention,
PE mask-strip accumulate, rstd via ln/exp (single activation table),
fp16 output writeback.
"""

import os

import numpy as np
from ml_dtypes import bfloat16 as np_bf16

import bass_rust as _bass_rust
import concourse.bass as bass
import concourse.mybir as mybir
import concourse.tile as tile
from concourse import bacc
from concourse import bass_utils
from concourse.hw_specs import get_activation_tables
from concourse.masks import make_identity


class _PinnedBacc(bacc.Bacc):
    """Bacc that pins all activations to one act-func table.

    Square/Ln/Exp/Copy all live in natural_log_exp_and_others; the default
    greedy table choice alternates tables (Ln -> natural_log, Exp ->
    exp_and_others), inserting a 1.3us table load per switch. Emptying the
    other tables makes the selection pass pick the shared table with its
    original act_info index, so exactly one load is emitted."""

    _PIN_TABLE = "natural_log_exp_and_others"

    def insert_act_table_loads(self):
        has_activation = any(
            isinstance(i, mybir.InstActivation)
            for b in self.main_func.blocks
            for i in b.instructions
        )
        if not has_activation:
            return
        tables = [(k, v if k == self._PIN_TABLE else set())
                  for k, v in get_activation_tables(self.m.arch).items()]
        _bass_rust.insert_act_table_loads(self, tables)

F32 = mybir.dt.float32
F16 = mybir.dt.float16
BF16 = mybir.dt.bfloat16
AF = mybir.ActivationFunctionType
ALU = mybir.AluOpType

B, L, D = 2, 2048, 2048
H, KV, HD = 16, 4, 64
EPS = 1e-6
ROPE_BASE = 10000.0
N_CORES = 8
GQ = H // KV          # 4 query heads per core
LT = L // 128         # 16 l-tiles
DT = D // 128         # 16 d-tiles (contraction tiles for qkv proj)
TQ = 512              # q-chunk width for attention
NQC = L // TQ         # 4 q-chunks
NKB = L // 128        # 16 k-blocks
EW = (GQ + 2) * HD    # 384 qkv channels per core
EO = GQ * HD          # 256 output channels per core
G5 = GQ + 1           # norm groups (4 q heads + 1 k head)

KOPT_SCHED = os.environ.get("KOPT_SCHED", "ilv")
KOPT_LEAD = int(os.environ.get("KOPT_LEAD", "2"))  # filler lead, fifths


def _classify_mask(mask):
    """Per (kb, qt) block: 'skip' | 'full' | pattern index into mixed list.

    Patterns are transposed slices maskT[k0:k0+128, q0:q0+TQ]."""
    kinds = {}
    patterns = []
    pat_ids = {}
    deltas = {}
    for qt in range(NQC):
        for kb in range(NKB):
            sub = mask[qt * TQ:(qt + 1) * TQ, kb * 128:(kb + 1) * 128]
            if np.all(sub <= -1e8):
                kinds[(kb, qt)] = "skip"
            elif np.all(sub == 0.0):
                kinds[(kb, qt)] = "full"
            else:
                pt = np.ascontiguousarray(sub.T.astype(np.float32))
                key = pt.tobytes()
                if key not in pat_ids:
                    pat_ids[key] = len(patterns)
                    patterns.append(pt)
                kinds[(kb, qt)] = pat_ids[key]
                # affine (causal-boundary) pattern? keep iff q >= k
                delta = kb * 128 - qt * TQ
                kk = np.arange(128)[:, None]
                qq = np.arange(TQ)[None, :]
                causal = np.where(qq >= kk + delta, 0.0, -1e9).astype(np.float32)
                deltas[(kb, qt)] = delta if np.array_equal(pt, causal) else None
    return kinds, patterns, deltas


def _build_program(kinds, n_mixed, repeat=1, deltas=None, W_FOLDED=False):
    nc = _PinnedBacc("TRN2", target_bir_lowering=False, debug=False,
                     enable_asserts=False, num_devices=N_CORES)
    deltas = deltas or {}
    # causal fast path: every mixed pattern has an affine delta
    causal = all(deltas.get(k) is not None
                 for k, v in kinds.items() if not isinstance(v, str))

    # DRAM I/O (per core). Host pre-tiles everything into DMA-friendly layouts.
    # xT[lt] is the exact [128, D] SBUF image: xT[lt][p, t*128+j] =
    # x[b, lt*128+j, t*128+p], so the load is one contiguous-line DMA.
    xT = nc.dram_tensor("xT", [LT, 128, D], BF16, kind="ExternalInput").ap()
    wqkT = nc.dram_tensor("wqkT", [D, EW], BF16, kind="ExternalInput").ap()
    woT = nc.dram_tensor("woT", [EO, D], BF16, kind="ExternalInput").ap()
    cos2 = nc.dram_tensor("cos2", [128, LT * 32], BF16, kind="ExternalInput").ap()
    sin2 = nc.dram_tensor("sin2", [128, LT * 32], BF16, kind="ExternalInput").ap()
    if not W_FOLDED:
        qw = nc.dram_tensor("qw", [128, G5 * HD], BF16, kind="ExternalInput").ap()
    if n_mixed:
        if causal:
            mblk = nc.dram_tensor("mblk", [128, n_mixed * 128], BF16,
                                  kind="ExternalInput").ap()
        else:
            mblk = nc.dram_tensor("mblk", [128, n_mixed * TQ], BF16,
                                  kind="ExternalInput").ap()
    y = nc.dram_tensor("y", [L, D], F16, kind="ExternalOutput").ap()

    with tile.TileContext(nc) as tc:
        with (
            tc.tile_pool(name="consts", bufs=1) as consts,
            tc.tile_pool(name="wpool", bufs=1) as wpool,
            tc.tile_pool(name="xcolp", bufs=10) as xcolp,
            tc.tile_pool(name="work", bufs=4) as work,
            tc.tile_pool(name="nrm", bufs=4) as nrm,
            tc.tile_pool(name="persist", bufs=1) as persist,
            tc.tile_pool(name="pp", bufs=10) as pp,
            tc.tile_pool(name="zp", bufs=3) as zp,
            tc.tile_pool(name="ps_a", bufs=2, space="PSUM") as ps_a,
            tc.tile_pool(name="ps_b", bufs=2, space="PSUM") as ps_b,
            tc.tile_pool(name="ps_sc", bufs=2, space="PSUM") as ps_sc,
        ):
            # ---- constants ----
            identf = consts.tile([128, 128], F32, tag="identf")
            make_identity(nc, identf[:])
            ident = consts.tile([128, 128], BF16, tag="ident")
            nc.vector.tensor_copy(ident[:], identf[:])
            cos_sb = consts.tile([128, LT * 32], BF16, tag="cos")
            sin_sb = consts.tile([128, LT * 32], BF16, tag="sin")
            nc.sync.dma_start(cos_sb[:], cos2[:])
            nc.sync.dma_start(sin_sb[:], sin2[:])
            # per-group ln() bias tiles for rstd = exp(-.5*ln(ss*s + b))
            # q groups: rstd = (w0q/sqrt(HD)) / sqrt(ss/HD + eps)
            #         = exp(-.5 * ln(ss * sq + bq))
            bq_sb = consts.tile([128, 1], F32, tag="bq")
            bk_sb = consts.tile([128, 1], F32, tag="bk")
            # constants depend on fold coefficients; host passes them via
            # module-level attributes set in _host_prep (W_FOLDED only).
            cq, ck = _FOLD_COEFS
            sq_scale = 1.0 / (HD * cq * cq)
            bq_val = EPS / (cq * cq)
            sk_scale = 1.0 / (HD * ck * ck)
            bk_val = EPS / (ck * ck)
            nc.vector.memset(bq_sb[:], bq_val)
            nc.vector.memset(bk_sb[:], bk_val)
            if not W_FOLDED:
                w5_sb = consts.tile([128, G5 * HD], BF16, tag="w5")
                nc.sync.dma_start(w5_sb[:], qw[:])
            if n_mixed:
                mwid = 128 if causal else TQ
                mb_sb = consts.tile([128, n_mixed * mwid], BF16, tag="mb")
                nc.sync.dma_start(mb_sb[:], mblk[:])

            # ---- weights (bf16) ----
            wqk_sb = []
            for dt_i in range(DT):
                w = wpool.tile([128, EW], BF16, tag=f"wqk{dt_i}")
                nc.sync.dma_start(w[:], wqkT[dt_i * 128:(dt_i + 1) * 128, :])
                wqk_sb.append(w)
            wo_sb = []
            for et in range(2):
                w = wpool.tile([128, D], BF16, tag=f"wo{et}")
                nc.sync.dma_start(w[:], woT[et * 128:(et + 1) * 128, :])
                wo_sb.append(w)

            # ---- persistent attention operands ----
            # Q^T head pairs stacked on partitions: qpair rows 0-63 = head 2i,
            # rows 64-127 = head 2i+1 (direct XBAR transpose layout).
            # K^T duplicated on both halves. V-hat [tok part, 64 v | 64 ones].
            qpair2 = persist.tile([128, 2 * L], BF16, tag="qpair2")
            kt_sb = persist.tile([128, L], BF16, tag="kt")
            vt_sb = persist.tile([128, LT * 128], BF16, tag="vt")
            ones_sb = consts.tile([128, HD], BF16, tag="ones")
            nc.vector.memset(ones_sb[:], 1.0)
            for i in range(LT):
                nc.vector.tensor_copy(
                    vt_sb[:, i * 128 + HD:(i + 1) * 128], ones_sb[:])
            aot_sb = [persist.tile([128, L], BF16, tag=f"aot{et}",
                                   name=f"aot{et}")
                      for et in range(2)]

            # ================= Phase 1: QKV + RMSNorm + RoPE =================
            xcols = {}

            def prefetch_x(lt):
                if lt >= LT or lt in xcols:
                    return
                xcol = xcolp.tile([128, D], BF16, tag="xcol")
                # SWDGE queue (gpsimd): keeps the SP queue free for the
                # latency-critical XBAR transposes.
                nc.gpsimd.dma_start(xcol[:], xT[lt, :, :])
                xcols[lt] = xcol

            def emit_p1(lt):
                prefetch_x(lt)
                xcol = xcols.pop(lt)
                prefetch_x(lt + 1)
                prefetch_x(lt + 2)
                qkv_ps = ps_a.tile([128, 512], F32, tag="mm_a")
                qk = qkv_ps[:, 0:EW]
                for dt_i in range(DT):
                    nc.tensor.matmul(
                        qk, xcol[:, dt_i * 128:(dt_i + 1) * 128],
                        wqk_sb[dt_i][:],
                        start=(dt_i == 0), stop=(dt_i == DT - 1))
                # V copy right away so the qkv PSUM slot frees early
                nc.vector.tensor_copy(
                    vt_sb[:, lt * 128:lt * 128 + HD],
                    qk[:, G5 * HD:(G5 + 1) * HD])
                # RMS stats for 5 norm groups (4 q heads + 1 k head):
                # one batched Square then a per-group DVE reduction
                sq5 = work.tile([128, G5 * HD], BF16, tag="sq5")
                nc.scalar.activation(sq5[:], qk[:, 0:G5 * HD], AF.Square)
                ss = nrm.tile([128, 16], F32, tag="ss")
                nc.vector.tensor_reduce(
                    ss[:, 0:G5],
                    sq5[:].rearrange("p (h e) -> p h e", e=HD),
                    axis=mybir.AxisListType.X, op=ALU.add)
                # rstd = exp(-.5*ln(ss*s + b)); same act table as Exp/Square
                nc.scalar.activation(ss[:, 8:8 + GQ], ss[:, 0:GQ],
                                     AF.Ln, bias=bq_sb[:], scale=sq_scale)
                nc.scalar.activation(ss[:, 8 + GQ:8 + G5], ss[:, GQ:G5],
                                     AF.Ln, bias=bk_sb[:], scale=sk_scale)
                rstd = nrm.tile([128, 8], F32, tag="rstd")
                nc.scalar.activation(rstd[:, 0:G5], ss[:, 8:8 + G5],
                                     AF.Exp, scale=-0.5)

                # normalize: qn = qkv * rstd (broadcast over head dim)
                qn = work.tile([128, G5 * HD], BF16, tag="qn")
                nc.vector.tensor_tensor(
                    qn[:].rearrange("p (h e) -> p h e", e=HD),
                    qk[:, 0:G5 * HD].rearrange("p (h e) -> p h e", e=HD),
                    rstd[:, 0:G5, None].broadcast_to([128, G5, HD]),
                    op=ALU.mult)
                if not W_FOLDED:
                    nc.vector.tensor_tensor(qn[:], qn[:], w5_sb[:],
                                            op=ALU.mult)

                # RoPE on all 5 groups at once (bf16, DVE 2x mode)
                cs = cos_sb[:, lt * 32:(lt + 1) * 32]
                sn = sin_sb[:, lt * 32:(lt + 1) * 32]
                csq = cs[:, None, :].broadcast_to([128, G5, 32])
                snq = sn[:, None, :].broadcast_to([128, G5, 32])
                # rq layout: q0 q1 q2 q3 k k2 (k duplicated for kt transpose)
                rq = work.tile([128, (G5 + 1) * HD], BF16, tag="rq")
                rqv = rq[:, 0:G5 * HD].rearrange("p (h e) -> p h e", e=HD)
                qnv = qn[:].rearrange("p (h e) -> p h e", e=HD)
                t1 = work.tile([128, G5 * 32], BF16, tag="t1")
                t1v = t1[:].rearrange("p (h e) -> p h e", e=32)
                # low half: x1*cos - x2*sin
                nc.vector.tensor_tensor(t1v, qnv[:, :, 0:32], csq, op=ALU.mult)
                nc.vector.tensor_tensor(rqv[:, :, 0:32], qnv[:, :, 32:64], snq,
                                        op=ALU.mult)
                nc.vector.tensor_tensor(rqv[:, :, 0:32], t1v,
                                        rqv[:, :, 0:32], op=ALU.subtract)
                # high half: x1*sin + x2*cos
                nc.vector.tensor_tensor(t1v, qnv[:, :, 0:32], snq, op=ALU.mult)
                nc.vector.tensor_tensor(rqv[:, :, 32:64], qnv[:, :, 32:64], csq,
                                        op=ALU.mult)
                nc.vector.tensor_tensor(rqv[:, :, 32:64], t1v,
                                        rqv[:, :, 32:64], op=ALU.add)
                # duplicate k so one XBAR op yields kt on both halves
                nc.vector.tensor_copy(rq[:, G5 * HD:(G5 + 1) * HD],
                                      rq[:, GQ * HD:G5 * HD])

                # transposes via XBAR DMA (no PE, no DVE)
                for pr in range(2):
                    nc.sync.dma_start_transpose(
                        qpair2[:, pr * L + lt * 128:pr * L + (lt + 1) * 128],
                        rq[:, pr * 128:(pr + 1) * 128])
                nc.sync.dma_start_transpose(
                    kt_sb[:, lt * 128:(lt + 1) * 128],
                    rq[:, GQ * HD:(GQ + 2) * HD])

            # ================= Phase 2: attention =================
            # Per (pair, qc): both sub-heads' scores/exp/AV pipeline over kb
            # with per-sub 1-bank PSUM tiles; AV of step kb-1 is emitted
            # after the scores of kb so PE always has ready work while the
            # exp for kb runs.
            def p2_steps(qc):
                """Yield emission closures for one q-chunk, software-pipelined."""
                klist = [kb for kb in range(NKB) if kinds[(kb, qc)] != "skip"]
                if not klist:
                    return

                for pr in range(2):
                    qsl = qpair2[:, pr * L + qc * TQ:pr * L + (qc + 1) * TQ]
                    avs = []    # allocated lazily at first step execution
                    pend = []   # (kb, d, p0, p1) awaiting AV

                    def alloc_avs(avs=avs, qc=qc, pr=pr):
                        if not avs:
                            for s in range(2):
                                avs.append(ps_b.tile(
                                    [128, TQ], F32, tag="av",
                                    name=f"av{qc}_{pr}_{s}"))

                    def flush_av(pend=pend, avs=avs, last=False):
                        while pend and (last or len(pend) > 1):
                            kb0, d0, p_sb = pend.pop(0)
                            fin = last and not pend
                            for sub in range(2):
                                nc.tensor.matmul(
                                    avs[sub][:, d0:TQ],
                                    vt_sb[:, kb0 * 128:(kb0 + 1) * 128],
                                    p_sb[:, sub * TQ + d0:(sub + 1) * TQ],
                                    start=kb0 == klist[0], stop=fin,
                                    skip_group_check=True)

                    def step(kb, pr=pr, qsl=qsl, avs=avs, pend=pend,
                             alloc_avs=alloc_avs, flush_av=flush_av):
                        alloc_avs()
                        kind = kinds[(kb, qc)]
                        delta = deltas.get((kb, qc))
                        if kind == "full" or not causal:
                            d = 0
                        else:
                            d = max(delta, 0)
                        sc_ps = ps_sc.tile([128, 2 * TQ], F32, tag="sc")
                        for sub in range(2):
                            nc.tensor.matmul(
                                sc_ps[:, sub * TQ + d:(sub + 1) * TQ],
                                kt_sb[sub * 64:(sub + 1) * 64,
                                      kb * 128:(kb + 1) * 128],
                                qsl[sub * 64:(sub + 1) * 64, d:TQ],
                                start=True, stop=(kind == "full"),
                                skip_group_check=True)
                        if kind != "full":
                            if causal:
                                m_mv = mb_sb[:, kind * 128:(kind + 1) * 128]
                                moff, mw = d, 128
                            else:
                                m_mv = mb_sb[:, kind * TQ:(kind + 1) * TQ]
                                moff, mw = 0, TQ
                            for sub in range(2):
                                nc.tensor.matmul(
                                    sc_ps[:, sub * TQ + moff:
                                          sub * TQ + moff + mw],
                                    ident[:], m_mv,
                                    start=False, stop=True,
                                    skip_group_check=True)
                        flush_av()
                        p_sb = pp.tile([128, 2 * TQ], BF16, tag="p")
                        sc_view = sc_ps[:].rearrange(
                            "p (s q) -> p s q", q=TQ)[:, :, d:TQ]
                        p_view = p_sb[:].rearrange(
                            "p (s q) -> p s q", q=TQ)[:, :, d:TQ]
                        nc.scalar.activation(p_view, sc_view, AF.Exp)
                        pend.append((kb, d, p_sb))

                    for kb in klist:
                        yield lambda kb=kb, step=step: step(kb)

                    def finish(pr=pr, avs=avs, pend=pend, flush_av=flush_av):
                        flush_av(last=True)
                        for sub in range(2):
                            rec = work.tile([64, TQ], F32, tag="rec")
                            nc.vector.reciprocal(rec[:], avs[sub][64:128, :])
                            nc.vector.tensor_tensor(
                                aot_sb[pr][sub * 64:(sub + 1) * 64,
                                           qc * TQ:(qc + 1) * TQ],
                                avs[sub][0:64, :], rec[:], op=ALU.mult)

                    yield finish

            # ================= Phase 3: output projection =================
            def emit_p3(lt):
                zo = zp.tile([128, D], F16, tag="zo")
                for dc in range(4):
                    z_ps = ps_a.tile([128, 512], F32, tag="mm_a")
                    for et in range(2):
                        nc.tensor.matmul(
                            z_ps[:], aot_sb[et][:, lt * 128:(lt + 1) * 128],
                            wo_sb[et][:, dc * 512:(dc + 1) * 512],
                            start=(et == 0), stop=(et == 1))
                    zslice = zo[:, dc * 512:(dc + 1) * 512]
                    nc.vector.tensor_copy(zslice, z_ps[:])
                nc.gpsimd.dma_start(y[lt * 128:(lt + 1) * 128, :], zo[:])

            def emit_body():
                if KOPT_SCHED == "seq":
                    for lt in range(LT):
                        emit_p1(lt)
                    for qc in range(NQC):
                        for s in p2_steps(qc):
                            s()
                    for lt in range(LT):
                        emit_p3(lt)
                else:
                    # interleaved: P1 tiles run ahead of P2 q-chunks; P3
                    # trails one q-chunk behind P2.
                    lt_per_qc = TQ // 128
                    for lt in range(lt_per_qc):
                        emit_p1(lt)
                    next_p1 = lt_per_qc
                    next_p3 = 0
                    for qc in range(NQC):
                        steps = list(p2_steps(qc))
                        p1f = []
                        p1_hi = min(LT, lt_per_qc * (qc + 3))
                        while next_p1 < p1_hi:
                            p1f.append(next_p1)
                            next_p1 += 1
                        p3f = []
                        p3_hi = lt_per_qc * qc
                        while next_p3 < p3_hi:
                            p3f.append(next_p3)
                            next_p3 += 1
                        # x loads for this superstep's P1 fillers issue up
                        # front; filler bodies go in the BACK half of the
                        # steps, where the PE runs out of exp-gated work.
                        for i in p1f:
                            prefetch_x(i)
                        fillers = ([("p1", i) for i in p1f]
                                   + [("p3", i) for i in p3f])
                        nf, ns = len(fillers), max(len(steps), 1)
                        lead = ns * KOPT_LEAD // 5
                        span = max(ns - lead, 1)
                        fi = 0
                        for si, s in enumerate(steps):
                            s()
                            want = max(si + 1 - lead, 0) * nf // span
                            while fi < min(want, nf):
                                kind, idx = fillers[fi]
                                (emit_p1 if kind == "p1" else emit_p3)(idx)
                                fi += 1
                        while fi < nf:
                            kind, idx = fillers[fi]
                            (emit_p1 if kind == "p1" else emit_p3)(idx)
                            fi += 1
                    while next_p3 < LT:
                        emit_p3(next_p3)
                        next_p3 += 1

            if repeat > 1:
                with tc.For_i(0, repeat, 1):
                    emit_body()
            else:
                emit_body()

    nc.compile()
    return nc


_PROGRAM_CACHE = {}
_FOLD_COEFS = (HD ** -0.5, 1.0)


def _get_program(kinds, n_mixed, repeat=1, deltas=None, W_FOLDED=False):
    key = (tuple(sorted(kinds.items())), n_mixed, repeat, W_FOLDED,
           _FOLD_COEFS, KOPT_SCHED)
    if key not in _PROGRAM_CACHE:
        _PROGRAM_CACHE[key] = _build_program(kinds, n_mixed, repeat, deltas,
                                             W_FOLDED)
    return _PROGRAM_CACHE[key]


def _host_prep(x, W_qkv, W_out, q_norm_w, k_norm_w, mask):
    global _FOLD_COEFS
    kinds, patterns, deltas = _classify_mask(np.asarray(mask))
    n_mixed = len(patterns)
    assert n_mixed <= 12, f"too many unique mask patterns: {n_mixed}"
    causal = all(deltas.get(k) is not None
                 for k, v in kinds.items() if not isinstance(v, str))

    # RoPE tables, tiled [128, LT*32]: cos2[p, lt*32+j] = cos((lt*128+p)*freq_j)
    j = np.arange(0, HD, 2, dtype=np.float32)
    freqs = (ROPE_BASE ** (-j / HD)).astype(np.float32)
    pos = np.arange(L, dtype=np.float32)
    theta = pos[:, None] * freqs[None, :]
    cosf = np.cos(theta).astype(np.float32)     # [L, 32]
    sinf = np.sin(theta).astype(np.float32)
    cos2 = np.ascontiguousarray(
        cosf.reshape(LT, 128, 32).transpose(1, 0, 2).reshape(128, LT * 32)
    ).astype(np_bf16)
    sin2 = np.ascontiguousarray(
        sinf.reshape(LT, 128, 32).transpose(1, 0, 2).reshape(128, LT * 32)
    ).astype(np_bf16)

    scale = np.float32(HD ** -0.5)
    qwv = np.asarray(q_norm_w, np.float32)
    kwv = np.asarray(k_norm_w, np.float32)
    # uniform norm weights fold into the rstd ln/exp constants
    w_folded = bool(np.all(qwv == qwv[0]) and np.all(kwv == kwv[0]))
    if w_folded:
        _FOLD_COEFS = (float(qwv[0]) * float(scale), float(kwv[0]))
        qw_rep = None
    else:
        _FOLD_COEFS = (float(scale), 1.0)
        w5 = np.concatenate([np.tile(qwv, GQ), kwv]).astype(np.float32)
        qw_rep = np.tile(w5[None, :], (128, 1)).astype(np_bf16)

    if n_mixed:
        if causal:
            strips = []
            for pi, pt in enumerate(patterns):
                dlist = [d for k, d in deltas.items()
                         if kinds.get(k) == pi and d is not None]
                d = max(dlist[0], 0)
                strips.append(pt[:, d:d + 128])
            mb = np.concatenate(strips, axis=1).astype(np_bf16)
        else:
            mb = np.concatenate(patterns, axis=1).astype(np_bf16)
    else:
        mb = None

    in_maps = []
    for c in range(N_CORES):
        b, g = divmod(c, KV)
        xb = np.asarray(x[b], np.float32)
        # [LT, 128, D]: xTt[lt, p, t*128+j] = x[b, lt*128+j, t*128+p]
        xTt = np.ascontiguousarray(
            xb.reshape(LT, 128, DT, 128)        # [lt, j, t, p]
            .transpose(0, 3, 2, 1)              # [lt, p, t, j]
            .reshape(LT, 128, D)).astype(np_bf16)
        rows = np.r_[g * GQ * HD:(g + 1) * GQ * HD,
                     (H + g) * HD:(H + g + 1) * HD,
                     (H + KV + g) * HD:(H + KV + g + 1) * HD]
        wqkT = np.ascontiguousarray(
            np.asarray(W_qkv, np.float32)[rows].T).astype(np_bf16)
        cols = np.arange(g * GQ * HD, (g + 1) * GQ * HD)
        woT = np.ascontiguousarray(
            np.asarray(W_out, np.float32)[:, cols].T).astype(np_bf16)
        m = {"xT": xTt, "wqkT": wqkT, "woT": woT,
             "cos2": cos2, "sin2": sin2}
        if qw_rep is not None:
            m["qw"] = qw_rep
        if mb is not None:
            m["mblk"] = mb
        in_maps.append(m)
    return kinds, n_mixed, in_maps, deltas, w_folded


def kernel(x, W_qkv, W_out, q_norm_w, k_norm_w, mask):
    kinds, n_mixed, in_maps, deltas, wf = _host_prep(x, W_qkv, W_out,
                                                     q_norm_w, k_norm_w, mask)
    nc = _get_program(kinds, n_mixed, deltas=deltas, W_FOLDED=wf)
    res = bass_utils.run_bass_kernel_spmd(nc, in_maps,
                                          core_ids=list(range(N_CORES)))
    out = np.zeros((B, L, D), dtype=np.float32)
    for c in range(N_CORES):
        b = c // KV
        out[b] += res.results[c]["y"].astype(np.float32)
    return out


# revision 42
# speedup vs baseline: 1.5107x; 1.1352x over previous
"""Trainium2 Bass kernel for fused GQA attention block (B=2, L=2048, D=2048,
H=16 q-heads, KV=4 kv-heads, HD=64, causal, QK-RMSNorm + RoPE).

Sharding (8 cores): core c -> batch b = c // 4, head-group g = c % 4
(query heads 4g..4g+3, kv head g). Each core computes its 4 heads'
attention and a partial output projection (256 of 1024 e-channels);
host sums the 4 partials per batch.

v2: bf16 matmul path, XBAR DMA transposes, causal q-slicing in attention,
PE mask-strip accumulate, rstd via ln/exp (single activation table),
fp16 output writeback.
"""

import os

import numpy as np
from ml_dtypes import bfloat16 as np_bf16

import bass_rust as _bass_rust
import concourse.bass as bass
import concourse.mybir as mybir
import concourse.tile as tile
from concourse import bacc
from concourse import bass_utils
from concourse.hw_specs import get_activation_tables
from concourse.masks import make_identity


class _PinnedBacc(bacc.Bacc):
    """Bacc that pins all activations to one act-func table.

    Square/Ln/Exp/Copy all live in natural_log_exp_and_others; the default
    greedy table choice alternates tables (Ln -> natural_log, Exp ->
    exp_and_others), inserting a 1.3us table load per switch. Emptying the
    other tables makes the selection pass pick the shared table with its
    original act_info index, so exactly one load is emitted."""

    _PIN_TABLE = "natural_log_exp_and_others"

    def insert_act_table_loads(self):
        has_activation = any(
            isinstance(i, mybir.InstActivation)
            for b in self.main_func.blocks
            for i in b.instructions
        )
        if not has_activation:
            return
        tables = [(k, v if k == self._PIN_TABLE else set())
                  for k, v in get_activation_tables(self.m.arch).items()]
        _bass_rust.insert_act_table_loads(self, tables)

F32 = mybir.dt.float32
F16 = mybir.dt.float16
BF16 = mybir.dt.bfloat16
AF = mybir.ActivationFunctionType
ALU = mybir.AluOpType

B, L, D = 2, 2048, 2048
H, KV, HD = 16, 4, 64
EPS = 1e-6
ROPE_BASE = 10000.0
N_CORES = 8
GQ = H // KV          # 4 query heads per core
LT = L // 128         # 16 l-tiles
DT = D // 128         # 16 d-tiles (contraction tiles for qkv proj)
TQ = 512              # q-chunk width for attention
NQC = L // TQ         # 4 q-chunks
NKB = L // 128        # 16 k-blocks
EW = (GQ + 2) * HD    # 384 qkv channels per core
EO = GQ * HD          # 256 output channels per core
G5 = GQ + 1           # norm groups (4 q heads + 1 k head)

KOPT_SCHED = os.environ.get("KOPT_SCHED", "ilv")
KOPT_LEAD = int(os.environ.get("KOPT_LEAD", "2"))  # filler lead, fifths


def _classify_mask(mask):
    """Per (kb, qt) block: 'skip' | 'full' | pattern index into mixed list.

    Patterns are transposed slices maskT[k0:k0+128, q0:q0+TQ]."""
    kinds = {}
    patterns = []
    pat_ids = {}
    deltas = {}
    for qt in range(NQC):
        for kb in range(NKB):
            sub = mask[qt * TQ:(qt + 1) * TQ, kb * 128:(kb + 1) * 128]
            if np.all(sub <= -1e8):
                kinds[(kb, qt)] = "skip"
            elif np.all(sub == 0.0):
                kinds[(kb, qt)] = "full"
            else:
                pt = np.ascontiguousarray(sub.T.astype(np.float32))
                key = pt.tobytes()
                if key not in pat_ids:
                    pat_ids[key] = len(patterns)
                    patterns.append(pt)
                kinds[(kb, qt)] = pat_ids[key]
                # affine (causal-boundary) pattern? keep iff q >= k
                delta = kb * 128 - qt * TQ
                kk = np.arange(128)[:, None]
                qq = np.arange(TQ)[None, :]
                causal = np.where(qq >= kk + delta, 0.0, -1e9).astype(np.float32)
                deltas[(kb, qt)] = delta if np.array_equal(pt, causal) else None
    return kinds, patterns, deltas


def _build_program(kinds, n_mixed, repeat=1, deltas=None, W_FOLDED=False):
    nc = _PinnedBacc("TRN2", target_bir_lowering=False, debug=False,
                     enable_asserts=False, num_devices=N_CORES)
    deltas = deltas or {}
    # causal fast path: every mixed pattern has an affine delta
    causal = all(deltas.get(k) is not None
                 for k, v in kinds.items() if not isinstance(v, str))

    # DRAM I/O (per core). Host pre-tiles everything into DMA-friendly layouts.
    # xT[lt] is the exact [128, D] SBUF image: xT[lt][p, t*128+j] =
    # x[b, lt*128+j, t*128+p], so the load is one contiguous-line DMA.
    xT = nc.dram_tensor("xT", [LT, 128, D], BF16, kind="ExternalInput").ap()
    wqkT = nc.dram_tensor("wqkT", [D, EW], BF16, kind="ExternalInput").ap()
    woT = nc.dram_tensor("woT", [EO, D], BF16, kind="ExternalInput").ap()
    cos2 = nc.dram_tensor("cos2", [128, LT * 32], BF16, kind="ExternalInput").ap()
    sin2 = nc.dram_tensor("sin2", [128, LT * 32], BF16, kind="ExternalInput").ap()
    if not W_FOLDED:
        qw = nc.dram_tensor("qw", [128, G5 * HD], BF16, kind="ExternalInput").ap()
    if n_mixed:
        if causal:
            mblk = nc.dram_tensor("mblk", [128, n_mixed * 128], BF16,
                                  kind="ExternalInput").ap()
        else:
            mblk = nc.dram_tensor("mblk", [128, n_mixed * TQ], BF16,
                                  kind="ExternalInput").ap()
    y = nc.dram_tensor("y", [L, D], F16, kind="ExternalOutput").ap()

    with tile.TileContext(nc) as tc:
        with (
            tc.tile_pool(name="consts", bufs=1) as consts,
            tc.tile_pool(name="wpool", bufs=1) as wpool,
            tc.tile_pool(name="xcolp", bufs=10) as xcolp,
            tc.tile_pool(name="work", bufs=4) as work,
            tc.tile_pool(name="nrm", bufs=4) as nrm,
            tc.tile_pool(name="persist", bufs=1) as persist,
            tc.tile_pool(name="pp", bufs=10) as pp,
            tc.tile_pool(name="zp", bufs=3) as zp,
            tc.tile_pool(name="ps_a", bufs=2, space="PSUM") as ps_a,
            tc.tile_pool(name="ps_b", bufs=2, space="PSUM") as ps_b,
            tc.tile_pool(name="ps_sc", bufs=2, space="PSUM") as ps_sc,
        ):
            # ---- constants ----
            identf = consts.tile([128, 128], F32, tag="identf")
            make_identity(nc, identf[:])
            ident = consts.tile([128, 128], BF16, tag="ident")
            nc.vector.tensor_copy(ident[:], identf[:])
            cos_sb = consts.tile([128, LT * 32], BF16, tag="cos")
            sin_sb = consts.tile([128, LT * 32], BF16, tag="sin")
            nc.sync.dma_start(cos_sb[:], cos2[:])
            nc.sync.dma_start(sin_sb[:], sin2[:])
            # per-group ln() bias tiles for rstd = exp(-.5*ln(ss*s + b))
            # q groups: rstd = (w0q/sqrt(HD)) / sqrt(ss/HD + eps)
            #         = exp(-.5 * ln(ss * sq + bq))
            bq_sb = consts.tile([128, 1], F32, tag="bq")
            bk_sb = consts.tile([128, 1], F32, tag="bk")
            # constants depend on fold coefficients; host passes them via
            # module-level attributes set in _host_prep (W_FOLDED only).
            cq, ck = _FOLD_COEFS
            sq_scale = 1.0 / (HD * cq * cq)
            bq_val = EPS / (cq * cq)
            sk_scale = 1.0 / (HD * ck * ck)
            bk_val = EPS / (ck * ck)
            nc.vector.memset(bq_sb[:], bq_val)
            nc.vector.memset(bk_sb[:], bk_val)
            if not W_FOLDED:
                w5_sb = consts.tile([128, G5 * HD], BF16, tag="w5")
                nc.sync.dma_start(w5_sb[:], qw[:])
            if n_mixed:
                mwid = 128 if causal else TQ
                mb_sb = consts.tile([128, n_mixed * mwid], BF16, tag="mb")
                nc.sync.dma_start(mb_sb[:], mblk[:])

            # ---- weights (bf16) ----
            wqk_sb = []
            for dt_i in range(DT):
                w = wpool.tile([128, EW], BF16, tag=f"wqk{dt_i}")
                nc.sync.dma_start(w[:], wqkT[dt_i * 128:(dt_i + 1) * 128, :])
                wqk_sb.append(w)
            wo_sb = []
            for et in range(2):
                w = wpool.tile([128, D], BF16, tag=f"wo{et}")
                nc.sync.dma_start(w[:], woT[et * 128:(et + 1) * 128, :])
                wo_sb.append(w)

            # ---- persistent attention operands ----
            # Q^T head pairs stacked on partitions: qpair rows 0-63 = head 2i,
            # rows 64-127 = head 2i+1 (direct XBAR transpose layout).
            # K^T duplicated on both halves. V-hat [tok part, 64 v | 64 ones].
            # vt free layout per k-tile: [ones(64) | V(64)] so the AV output
            # puts softmax denominators on partitions 0-63 (the custom-DVE
            # fast reciprocal requires base partition 0).
            qpair2 = persist.tile([128, 2 * L], BF16, tag="qpair2")
            kt_sb = persist.tile([128, L], BF16, tag="kt")
            vt_sb = persist.tile([128, LT * 128], BF16, tag="vt")
            ones_sb = consts.tile([128, HD], BF16, tag="ones")
            nc.vector.memset(ones_sb[:], 1.0)
            for i in range(LT):
                nc.vector.tensor_copy(
                    vt_sb[:, i * 128:i * 128 + HD], ones_sb[:])
            aot_sb = [persist.tile([128, L], BF16, tag=f"aot{et}",
                                   name=f"aot{et}")
                      for et in range(2)]

            # ================= Phase 1: QKV + RMSNorm + RoPE =================
            xcols = {}

            def prefetch_x(lt):
                if lt >= LT or lt in xcols:
                    return
                xcol = xcolp.tile([128, D], BF16, tag="xcol")
                # SWDGE queue (gpsimd): keeps the SP queue free for the
                # latency-critical XBAR transposes.
                nc.gpsimd.dma_start(xcol[:], xT[lt, :, :])
                xcols[lt] = xcol

            def emit_p1(lt):
                prefetch_x(lt)
                xcol = xcols.pop(lt)
                prefetch_x(lt + 1)
                prefetch_x(lt + 2)
                qkv_ps = ps_a.tile([128, 512], F32, tag="mm_a")
                qk = qkv_ps[:, 0:EW]
                for dt_i in range(DT):
                    nc.tensor.matmul(
                        qk, xcol[:, dt_i * 128:(dt_i + 1) * 128],
                        wqk_sb[dt_i][:],
                        start=(dt_i == 0), stop=(dt_i == DT - 1))
                # V copy right away so the qkv PSUM slot frees early
                nc.vector.tensor_copy(
                    vt_sb[:, lt * 128 + HD:(lt + 1) * 128],
                    qk[:, G5 * HD:(G5 + 1) * HD])
                # RMS stats for 5 norm groups (4 q heads + 1 k head):
                # one batched Square then a per-group DVE reduction
                sq5 = work.tile([128, G5 * HD], BF16, tag="sq5")
                nc.scalar.activation(sq5[:], qk[:, 0:G5 * HD], AF.Square)
                ss = nrm.tile([128, 16], F32, tag="ss")
                nc.vector.tensor_reduce(
                    ss[:, 0:G5],
                    sq5[:].rearrange("p (h e) -> p h e", e=HD),
                    axis=mybir.AxisListType.X, op=ALU.add)
                # rstd = exp(-.5*ln(ss*s + b)); same act table as Exp/Square
                nc.scalar.activation(ss[:, 8:8 + GQ], ss[:, 0:GQ],
                                     AF.Ln, bias=bq_sb[:], scale=sq_scale)
                nc.scalar.activation(ss[:, 8 + GQ:8 + G5], ss[:, GQ:G5],
                                     AF.Ln, bias=bk_sb[:], scale=sk_scale)
                rstd = nrm.tile([128, 8], F32, tag="rstd")
                nc.scalar.activation(rstd[:, 0:G5], ss[:, 8:8 + G5],
                                     AF.Exp, scale=-0.5)

                # normalize: qn = qkv * rstd (broadcast over head dim)
                qn = work.tile([128, G5 * HD], BF16, tag="qn")
                nc.vector.tensor_tensor(
                    qn[:].rearrange("p (h e) -> p h e", e=HD),
                    qk[:, 0:G5 * HD].rearrange("p (h e) -> p h e", e=HD),
                    rstd[:, 0:G5, None].broadcast_to([128, G5, HD]),
                    op=ALU.mult)
                if not W_FOLDED:
                    nc.vector.tensor_tensor(qn[:], qn[:], w5_sb[:],
                                            op=ALU.mult)

                # RoPE on all 5 groups at once (bf16, DVE 2x mode)
                cs = cos_sb[:, lt * 32:(lt + 1) * 32]
                sn = sin_sb[:, lt * 32:(lt + 1) * 32]
                csq = cs[:, None, :].broadcast_to([128, G5, 32])
                snq = sn[:, None, :].broadcast_to([128, G5, 32])
                # rq layout: q0 q1 q2 q3 k k2 (k duplicated for kt transpose)
                rq = work.tile([128, (G5 + 1) * HD], BF16, tag="rq")
                rqv = rq[:, 0:G5 * HD].rearrange("p (h e) -> p h e", e=HD)
                qnv = qn[:].rearrange("p (h e) -> p h e", e=HD)
                t1 = work.tile([128, G5 * 32], BF16, tag="t1")
                t1v = t1[:].rearrange("p (h e) -> p h e", e=32)
                # low half: x1*cos - x2*sin
                nc.vector.tensor_tensor(t1v, qnv[:, :, 0:32], csq, op=ALU.mult)
                nc.vector.tensor_tensor(rqv[:, :, 0:32], qnv[:, :, 32:64], snq,
                                        op=ALU.mult)
                nc.vector.tensor_tensor(rqv[:, :, 0:32], t1v,
                                        rqv[:, :, 0:32], op=ALU.subtract)
                # high half: x1*sin + x2*cos
                nc.vector.tensor_tensor(t1v, qnv[:, :, 0:32], snq, op=ALU.mult)
                nc.vector.tensor_tensor(rqv[:, :, 32:64], qnv[:, :, 32:64], csq,
                                        op=ALU.mult)
                nc.vector.tensor_tensor(rqv[:, :, 32:64], t1v,
                                        rqv[:, :, 32:64], op=ALU.add)
                # duplicate k so one XBAR op yields kt on both halves
                nc.vector.tensor_copy(rq[:, G5 * HD:(G5 + 1) * HD],
                                      rq[:, GQ * HD:G5 * HD])

                # transposes via XBAR DMA (no PE, no DVE)
                for pr in range(2):
                    nc.sync.dma_start_transpose(
                        qpair2[:, pr * L + lt * 128:pr * L + (lt + 1) * 128],
                        rq[:, pr * 128:(pr + 1) * 128])
                nc.sync.dma_start_transpose(
                    kt_sb[:, lt * 128:(lt + 1) * 128],
                    rq[:, GQ * HD:(GQ + 2) * HD])

            # ================= Phase 2: attention =================
            # Per (pair, qc): both sub-heads' scores/exp/AV pipeline over kb
            # with per-sub 1-bank PSUM tiles; AV of step kb-1 is emitted
            # after the scores of kb so PE always has ready work while the
            # exp for kb runs.
            def p2_steps(qc):
                """Yield emission closures for one q-chunk, software-pipelined."""
                klist = [kb for kb in range(NKB) if kinds[(kb, qc)] != "skip"]
                if not klist:
                    return

                for pr in range(2):
                    qsl = qpair2[:, pr * L + qc * TQ:pr * L + (qc + 1) * TQ]
                    avs = []    # allocated lazily at first step execution
                    pend = []   # (kb, d, p0, p1) awaiting AV

                    def alloc_avs(avs=avs, qc=qc, pr=pr):
                        if not avs:
                            for s in range(2):
                                avs.append(ps_b.tile(
                                    [128, TQ], F32, tag="av",
                                    name=f"av{qc}_{pr}_{s}"))

                    def flush_av(pend=pend, avs=avs, last=False):
                        while pend and (last or len(pend) > 1):
                            kb0, d0, p_sb = pend.pop(0)
                            fin = last and not pend
                            for sub in range(2):
                                nc.tensor.matmul(
                                    avs[sub][:, d0:TQ],
                                    vt_sb[:, kb0 * 128:(kb0 + 1) * 128],
                                    p_sb[:, sub * TQ + d0:(sub + 1) * TQ],
                                    start=kb0 == klist[0], stop=fin,
                                    skip_group_check=True)

                    def step(kb, pr=pr, qsl=qsl, avs=avs, pend=pend,
                             alloc_avs=alloc_avs, flush_av=flush_av):
                        alloc_avs()
                        kind = kinds[(kb, qc)]
                        delta = deltas.get((kb, qc))
                        if kind == "full" or not causal:
                            d = 0
                        else:
                            d = max(delta, 0)
                        sc_ps = ps_sc.tile([128, 2 * TQ], F32, tag="sc")
                        for sub in range(2):
                            nc.tensor.matmul(
                                sc_ps[:, sub * TQ + d:(sub + 1) * TQ],
                                kt_sb[sub * 64:(sub + 1) * 64,
                                      kb * 128:(kb + 1) * 128],
                                qsl[sub * 64:(sub + 1) * 64, d:TQ],
                                start=True, stop=(kind == "full"),
                                skip_group_check=True)
                        if kind != "full":
                            if causal:
                                m_mv = mb_sb[:, kind * 128:(kind + 1) * 128]
                                moff, mw = d, 128
                            else:
                                m_mv = mb_sb[:, kind * TQ:(kind + 1) * TQ]
                                moff, mw = 0, TQ
                            for sub in range(2):
                                nc.tensor.matmul(
                                    sc_ps[:, sub * TQ + moff:
                                          sub * TQ + moff + mw],
                                    ident[:], m_mv,
                                    start=False, stop=True,
                                    skip_group_check=True)
                        flush_av()
                        p_sb = pp.tile([128, 2 * TQ], BF16, tag="p")
                        sc_view = sc_ps[:].rearrange(
                            "p (s q) -> p s q", q=TQ)[:, :, d:TQ]
                        p_view = p_sb[:].rearrange(
                            "p (s q) -> p s q", q=TQ)[:, :, d:TQ]
                        nc.scalar.activation(p_view, sc_view, AF.Exp)
                        pend.append((kb, d, p_sb))

                    for kb in klist:
                        yield lambda kb=kb, step=step: step(kb)

                    def finish(pr=pr, avs=avs, pend=pend, flush_av=flush_av):
                        flush_av(last=True)
                        for sub in range(2):
                            # softmax denominators are well away from the
                            # 0/denorm/inf edge cases, so the ~18-bit fast
                            # reciprocal (5x the plain one) is plenty.
                            rec = work.tile([64, TQ], F32, tag="rec")
                            nc.vector.reciprocal_approx_fast(
                                rec[:], avs[sub][0:64, :])
                            nc.vector.tensor_tensor(
                                aot_sb[pr][sub * 64:(sub + 1) * 64,
                                           qc * TQ:(qc + 1) * TQ],
                                avs[sub][64:128, :], rec[:], op=ALU.mult)

                    yield finish

            # ================= Phase 3: output projection =================
            def emit_p3(lt):
                zo = zp.tile([128, D], F16, tag="zo")
                for dc in range(4):
                    z_ps = ps_a.tile([128, 512], F32, tag="mm_a")
                    for et in range(2):
                        nc.tensor.matmul(
                            z_ps[:], aot_sb[et][:, lt * 128:(lt + 1) * 128],
                            wo_sb[et][:, dc * 512:(dc + 1) * 512],
                            start=(et == 0), stop=(et == 1))
                    zslice = zo[:, dc * 512:(dc + 1) * 512]
                    nc.vector.tensor_copy(zslice, z_ps[:])
                nc.gpsimd.dma_start(y[lt * 128:(lt + 1) * 128, :], zo[:])

            def emit_body():
                if KOPT_SCHED == "seq":
                    for lt in range(LT):
                        emit_p1(lt)
                    for qc in range(NQC):
                        for s in p2_steps(qc):
                            s()
                    for lt in range(LT):
                        emit_p3(lt)
                else:
                    # interleaved: P1 tiles run ahead of P2 q-chunks; P3
                    # trails one q-chunk behind P2.
                    lt_per_qc = TQ // 128
                    for lt in range(lt_per_qc):
                        emit_p1(lt)
                    next_p1 = lt_per_qc
                    next_p3 = 0
                    for qc in range(NQC):
                        steps = list(p2_steps(qc))
                        p1f = []
                        p1_hi = min(LT, lt_per_qc * (qc + 3))
                        while next_p1 < p1_hi:
                            p1f.append(next_p1)
                            next_p1 += 1
                        p3f = []
                        p3_hi = lt_per_qc * qc
                        while next_p3 < p3_hi:
                            p3f.append(next_p3)
                            next_p3 += 1
                        # x loads for this superstep's P1 fillers issue up
                        # front; filler bodies go in the BACK half of the
                        # steps, where the PE runs out of exp-gated work.
                        for i in p1f:
                            prefetch_x(i)
                        fillers = ([("p1", i) for i in p1f]
                                   + [("p3", i) for i in p3f])
                        nf, ns = len(fillers), max(len(steps), 1)
                        lead = ns * KOPT_LEAD // 5
                        span = max(ns - lead, 1)
                        fi = 0
                        for si, s in enumerate(steps):
                            s()
                            want = max(si + 1 - lead, 0) * nf // span
                            while fi < min(want, nf):
                                kind, idx = fillers[fi]
                                (emit_p1 if kind == "p1" else emit_p3)(idx)
                                fi += 1
                        while fi < nf:
                            kind, idx = fillers[fi]
                            (emit_p1 if kind == "p1" else emit_p3)(idx)
                            fi += 1
                    while next_p3 < LT:
                        emit_p3(next_p3)
                        next_p3 += 1

            if repeat > 1:
                with tc.For_i(0, repeat, 1):
                    emit_body()
            else:
                emit_body()

    nc.compile()
    return nc


_PROGRAM_CACHE = {}
_FOLD_COEFS = (HD ** -0.5, 1.0)


def _get_program(kinds, n_mixed, repeat=1, deltas=None, W_FOLDED=False):
    key = (tuple(sorted(kinds.items())), n_mixed, repeat, W_FOLDED,
           _FOLD_COEFS, KOPT_SCHED)
    if key not in _PROGRAM_CACHE:
        _PROGRAM_CACHE[key] = _build_program(kinds, n_mixed, repeat, deltas,
                                             W_FOLDED)
    return _PROGRAM_CACHE[key]


def _host_prep(x, W_qkv, W_out, q_norm_w, k_norm_w, mask):
    global _FOLD_COEFS
    kinds, patterns, deltas = _classify_mask(np.asarray(mask))
    n_mixed = len(patterns)
    assert n_mixed <= 12, f"too many unique mask patterns: {n_mixed}"
    causal = all(deltas.get(k) is not None
                 for k, v in kinds.items() if not isinstance(v, str))

    # RoPE tables, tiled [128, LT*32]: cos2[p, lt*32+j] = cos((lt*128+p)*freq_j)
    j = np.arange(0, HD, 2, dtype=np.float32)
    freqs = (ROPE_BASE ** (-j / HD)).astype(np.float32)
    pos = np.arange(L, dtype=np.float32)
    theta = pos[:, None] * freqs[None, :]
    cosf = np.cos(theta).astype(np.float32)     # [L, 32]
    sinf = np.sin(theta).astype(np.float32)
    cos2 = np.ascontiguousarray(
        cosf.reshape(LT, 128, 32).transpose(1, 0, 2).reshape(128, LT * 32)
    ).astype(np_bf16)
    sin2 = np.ascontiguousarray(
        sinf.reshape(LT, 128, 32).transpose(1, 0, 2).reshape(128, LT * 32)
    ).astype(np_bf16)

    scale = np.float32(HD ** -0.5)
    qwv = np.asarray(q_norm_w, np.float32)
    kwv = np.asarray(k_norm_w, np.float32)
    # uniform norm weights fold into the rstd ln/exp constants
    w_folded = bool(np.all(qwv == qwv[0]) and np.all(kwv == kwv[0]))
    if w_folded:
        _FOLD_COEFS = (float(qwv[0]) * float(scale), float(kwv[0]))
        qw_rep = None
    else:
        _FOLD_COEFS = (float(scale), 1.0)
        w5 = np.concatenate([np.tile(qwv, GQ), kwv]).astype(np.float32)
        qw_rep = np.tile(w5[None, :], (128, 1)).astype(np_bf16)

    if n_mixed:
        if causal:
            strips = []
            for pi, pt in enumerate(patterns):
                dlist = [d for k, d in deltas.items()
                         if kinds.get(k) == pi and d is not None]
                d = max(dlist[0], 0)
                strips.append(pt[:, d:d + 128])
            mb = np.concatenate(strips, axis=1).astype(np_bf16)
        else:
            mb = np.concatenate(patterns, axis=1).astype(np_bf16)
    else:
        mb = None

    in_maps = []
    for c in range(N_CORES):
        b, g = divmod(c, KV)
        xb = np.asarray(x[b], np.float32)
        # [LT, 128, D]: xTt[lt, p, t*128+j] = x[b, lt*128+j, t*128+p]
        xTt = np.ascontiguousarray(
            xb.reshape(LT, 128, DT, 128)        # [lt, j, t, p]
            .transpose(0, 3, 2, 1)              # [lt, p, t, j]
            .reshape(LT, 128, D)).astype(np_bf16)
        rows = np.r_[g * GQ * HD:(g + 1) * GQ * HD,
                     (H + g) * HD:(H + g + 1) * HD,
                     (H + KV + g) * HD:(H + KV + g + 1) * HD]
        wqkT = np.ascontiguousarray(
            np.asarray(W_qkv, np.float32)[rows].T).astype(np_bf16)
        cols = np.arange(g * GQ * HD, (g + 1) * GQ * HD)
        woT = np.ascontiguousarray(
            np.asarray(W_out, np.float32)[:, cols].T).astype(np_bf16)
        m = {"xT": xTt, "wqkT": wqkT, "woT": woT,
             "cos2": cos2, "sin2": sin2}
        if qw_rep is not None:
            m["qw"] = qw_rep
        if mb is not None:
            m["mblk"] = mb
        in_maps.append(m)
    return kinds, n_mixed, in_maps, deltas, w_folded


def kernel(x, W_qkv, W_out, q_norm_w, k_norm_w, mask):
    kinds, n_mixed, in_maps, deltas, wf = _host_prep(x, W_qkv, W_out,
                                                     q_norm_w, k_norm_w, mask)
    nc = _get_program(kinds, n_mixed, deltas=deltas, W_FOLDED=wf)
    res = bass_utils.run_bass_kernel_spmd(nc, in_maps,
                                          core_ids=list(range(N_CORES)))
    out = np.zeros((B, L, D), dtype=np.float32)
    for c in range(N_CORES):
        b = c // KV
        out[b] += res.results[c]["y"].astype(np.float32)
    return out
